# revision 1
# baseline (speedup 1.0000x reference)
"""DiffiT transformer block kernel for 8 Trainium2 NeuronCores.

Data-parallel over the B=64 window axis (8 windows per core). Activations
are feature-major ([channel, token]) so every linear contracts over the
SBUF partition axis. Q/K stay feature-major with heads packed at a 96-row
stride (so each head's 72 rows sit at 32-aligned partition bases and the
per-head score matmuls can slice them legally); V is produced token-major
into per-head slots with an appended ones-column, so O^T = V_aug.T @ P^T
yields the softmax denominator as row 72. Per-token scalars (LN mean/rstd,
softmax 1/l) are broadcast across partitions with K=1 ones-matmuls on the
PE. Dense matmuls run bf16; the residual stream stays fp32; small fixup
matmuls use float32r (full-rate fp32 at free-dim >= 256).

All biases and the time-token conditioning (c @ qkvt^T + biases) enter as
rank-1 (K=1) matmul fixups folded into the PSUM accumulations.
"""

import math
from contextlib import ExitStack

import numpy as np
import ml_dtypes

import concourse.bass as bass
import concourse.mybir as mybir
import concourse.tile as tile
from concourse import bacc
from concourse import bass_utils

F32 = mybir.dt.float32
F32R = mybir.dt.float32r
BF16 = mybir.dt.bfloat16
NPBF16 = ml_dtypes.bfloat16
AF = mybir.ActivationFunctionType

P = 128
WS = 16
N = 256            # tokens per window
C = 1152           # hidden
H = 16             # heads
DH = 72            # head dim
HS = 96            # head stride in the QK packing (32-aligned, >= DH)
MLP = 4608
EPS = 1e-6
B = 64
NCORES = 8
NW = B // NCORES   # windows per core
KC = C // P        # 9  k-tiles over the hidden dim
QKM = 2 * H * HS // P   # 24 m-tiles over packed Q+K (96-stride)
KOFF = QKM // 2    # first K-side m-tile
M1T = MLP // P     # 36 fc1 row tiles
SCALE = 1.0 / math.sqrt(DH)


def _r(ap):
    """view a 4-byte fp32 AP as float32r for full-rate PE matmuls"""
    return ap.bitcast(F32R)


def _qk_pieces(h):
    """32-aligned partition pieces covering head h's 72 rows in the
    96-stride packing: [(subtile, base, length), ...]; piece legality:
    base 0 any len, base 64 len<=64, base 32/96 len<=32."""
    start, end = HS * h, HS * h + DH
    out = []
    while start < end:
        sub, base = divmod(start, P)
        ln = min(end - start, P - base)
        if base == 64:
            ln = min(ln, 64)
        elif base in (32, 96):
            ln = min(ln, 32)
        elif base != 0:
            raise AssertionError(base)
        out.append((sub, base, ln))
        start += ln
    return out


def build_program(nw=NW, sim_gelu=False):
    nc = bacc.Bacc("TRN2", target_bir_lowering=False, debug=False,
                   num_devices=NCORES)

    # register the layernorm epsilon as a const AP (activation float biases
    # other than 0.0/1.0 need one), same pattern as Bass.__init__
    eps_t = nc.alloc_sbuf_tensor("const-eps", [P, 1], F32)
    nc.gpsimd.memset(eps_t.ap(), EPS)
    nc.const_aps.aps[(F32, EPS)] = eps_t.ap()
    nc.all_engine_barrier()

    def din(name, shape, dt):
        return nc.dram_tensor(name, shape, dt, kind="ExternalInput").ap()

    xT = din("xT", [nw, P, KC, N], F32)          # x, feature-major
    xTb = din("xTb", [nw, P, KC, N], BF16)       # x, bf16 copy for LN stats
    cT = din("cT", [10, P, nw], BF16)            # c augmented with ones row
    wct = din("wct", [10, P, 4224], BF16)        # qkvt^T reordered + bias row
    wqk = din("wqk", [QKM, P, KC, P], BF16)      # qkv^T QK part, 96-stride
    wv = din("wv", [4, P, KC, 288], BF16)        # qkv^T V part, chunk-major
    expb = din("expb", [H, P, 2, N], BF16)       # exp(rel-pos bias)^T per head
    wps = din("wps", [KC, P, H, P], BF16)        # proj^T, head-slot padded
    w1c = din("w1c", [M1T, P, KC, P], BF16)      # fc1^T pre-chunked
    w2 = din("w2", [KC, P, M1T, P], BF16)        # fc2^T, pm-chunked
    f1b = din("f1b", [P, M1T], F32)              # fc1 bias, per-partition
    b2 = din("b2", [1, 2 * C], BF16)             # proj_b ++ fc2_b
    outT = nc.dram_tensor("outT", [nw, P, KC, N], F32,
                          kind="ExternalOutput").ap()

    NPAIR = nw // 2
    W2N = 2 * N        # tokens per window pair

    with tile.TileContext(nc) as tc, ExitStack() as ctx:
        keep = ctx.enter_context(tc.tile_pool(name="keep", bufs=1))
        dram = ctx.enter_context(tc.tile_pool(name="dram", bufs=1,
                                              space="DRAM"))

        ones_b = keep.tile([1, W2N], BF16, tag="ones_b")  # bf16 rhs of K=1
        ones_c = keep.tile([P, 1], BF16, tag="ones_c")    # lhsT of column sums
        nc.gpsimd.memset(ones_b[:], 1.0)
        nc.gpsimd.memset(ones_c[:], 1.0)
        bias2 = keep.tile([1, 2 * C], BF16, tag="bias2")
        nc.sync.dma_start(bias2[:], b2[:])
        f1bs = keep.tile([P, M1T], F32, tag="f1bs")
        nc.sync.dma_start(f1bs[:], f1b[:])

        tdram = dram.tile([nw, 4224], BF16)
        xpd = dram.tile([nw, P, KC, N], F32)     # x after attention branch
        xpdb = dram.tile([nw, P, KC, N], BF16)   # bf16 shadow for LN2

        # ---- phase 0: conditioning T = c_aug @ W_ct ----------------------
        with tc.tile_pool(name="ph0", bufs=2) as p0, \
             tc.tile_pool(name="ph0p", bufs=2, space="PSUM") as pp0:
            caug = p0.tile([P, 10, nw], BF16, tag="caug")
            nc.sync.dma_start(caug[:], cT.rearrange("k p w -> p k w"))
            tsb = p0.tile([8, 4224], BF16, tag="tsb")
            for i in range(9):
                n0, nl = i * 512, min(512, 4224 - i * 512)
                tps = pp0.tile([8, 512], F32, tag="tps")
                for k in range(10):
                    wt = p0.tile([P, 512], BF16, tag="wctt")
                    nc.sync.dma_start(wt[:, :nl], wct[k, :, n0:n0 + nl])
                    nc.tensor.matmul(tps[:nw, :nl], caug[:, k, :], wt[:, :nl],
                                     start=(k == 0), stop=(k == 9))
                nc.scalar.activation(tsb[:nw, n0:n0 + nl], tps[:nw, :nl],
                                     AF.Copy)
            nc.sync.dma_start(tdram[:, :], tsb[:nw, :])

        # ---- layernorm for a window pair -> PSUM broadcast [P, W2N] ------
        # acc-tile layout: [:, :N]+[:, N:] hold the two windows; returns one
        # [P, W2N] psum tile pair (rstd bcast, -mean*rstd bcast)
        def ln_pair(pool, rows, accp, fetch, tag):
            """fetch(s) -> [P, W2N] bf16 tile of the LN input, sub-tile s.
            Returns (rstd_bcast, -mean*rstd bcast) PSUM tiles."""
            ms0 = accp.tile([1, W2N], F32, tag="acc")
            ms1 = accp.tile([1, W2N], F32, tag="acc")
            for s in range(KC):
                xbs = fetch(s)
                xsq = pool.tile([P, W2N], BF16, tag=tag + "xsq")
                nc.vector.tensor_mul(xsq[:], xbs[:], xbs[:])
                nc.tensor.matmul(ms0[:], ones_c[:], xbs[:],
                                 start=(s == 0), stop=(s == KC - 1))
                nc.tensor.matmul(ms1[:], ones_c[:], xsq[:],
                                 start=(s == 0), stop=(s == KC - 1))
            mean = rows.tile([1, W2N], F32, tag="r_mean")
            ra = rows.tile([1, W2N], F32, tag="r_a")
            rb = rows.tile([1, W2N], F32, tag="r_b")
            nc.vector.tensor_scalar_mul(mean[:], ms0[:], 1.0 / C)
            nc.vector.tensor_scalar_mul(ra[:], ms1[:], 1.0 / C)   # E[x^2]
            nc.vector.tensor_mul(rb[:], mean[:], mean[:])         # mean^2
            nc.vector.tensor_sub(ra[:], ra[:], rb[:])             # var
            nc.scalar.activation(rb[:], ra[:], AF.Sqrt, bias=EPS) # sd
            nc.vector.reciprocal_approx_fast(ra[:], rb[:])        # 1/sd
            rstd = rows.tile([1, W2N], BF16, tag="r_rstd")
            nc.gpsimd.tensor_copy(rstd[:], ra[:])
            bneg = rows.tile([1, W2N], BF16, tag="r_bneg")
            nc.vector.scalar_tensor_tensor(
                bneg[:], mean[:], -1.0, rstd[:],
                mybir.AluOpType.mult, mybir.AluOpType.mult)
            bc = accp.tile([P, W2N], F32, tag="acc")
            nc.tensor.matmul(bc[:], ones_b[:1, :P], rstd[:],
                             start=True, stop=True)
            bb = accp.tile([P, W2N], F32, tag="acc")
            nc.tensor.matmul(bb[:], ones_b[:1, :P], bneg[:],
                             start=True, stop=True)
            return bc, bb

        # ==== attention superphase: per pair LN1 -> QKV -> attn -> proj ===
        with tc.tile_pool(name="sp", bufs=2) as sp, \
             tc.tile_pool(name="sp1", bufs=1) as sp1, \
             tc.tile_pool(name="spw", bufs=2) as spw, \
             tc.tile_pool(name="sps", bufs=3) as sps, \
             tc.tile_pool(name="spr", bufs=2) as spr, \
             tc.tile_pool(name="rows", bufs=1) as rows, \
             tc.tile_pool(name="accp", bufs=8, space="PSUM") as accp:
            def fetch_dram_bf16(src, w0, pool, tag):
                def fetch(s):
                    t = pool.tile([P, W2N], BF16, tag=tag)
                    for wh in range(2):
                        nc.sync.dma_start(t[:, wh * N:(wh + 1) * N],
                                          src[w0 + wh, :, s, :])
                    return t
                return fetch

            for pr in range(NPAIR):
                w0 = 2 * pr
                f_x = fetch_dram_bf16(xTb, w0, spw, "xbs")
                bc, bb = ln_pair(spw, rows, accp, f_x, "ln1")
                hw = sp.tile([P, KC, W2N], BF16, tag="hw")
                for s in range(KC):
                    xbs = f_x(s)
                    nc.vector.tensor_mul(hw[:, s, :], xbs[:], bc[:])
                    nc.vector.tensor_add(hw[:, s, :], hw[:, s, :], bb[:])
                # QK (96-stride packed), N = both windows
                qkst = sp1.tile([P, QKM, W2N], BF16, tag="qkst")
                for m in range(QKM):
                    wt = spw.tile([P, KC, P], BF16, tag="wqkt")
                    nc.sync.dma_start(wt[:], wqk[m])
                    t1m = spw.tile([1, 2, P], BF16, tag="t1m")
                    nc.sync.dma_start(
                        t1m[:], tdram[w0:w0 + 2, P * m:P * (m + 1)]
                        .unsqueeze(0))
                    qs = accp.tile([P, W2N], F32, tag="acc")
                    for k in range(KC):
                        nc.tensor.matmul(qs[:], wt[:, k, :], hw[:, k, :],
                                         start=(k == 0), stop=False)
                    nc.tensor.matmul(qs[:, :N], t1m[:1, 0, :],
                                     ones_b[:1, :N], start=False, stop=False)
                    nc.tensor.matmul(qs[:, N:], t1m[:1, 1, :],
                                     ones_b[:1, :N], start=False, stop=True)
                    nc.scalar.activation(qkst[:, m, :], qs[:], AF.Copy)
                # V token-major into per-head slots (ones in col 0)
                vsl = sp1.tile([P, 2, 2, H, 73], BF16, tag="vsl")
                nc.vector.memset(vsl[:, :, :, :, 0:1], 1.0)
                for nch in range(4):
                    wvt = spw.tile([P, KC, 288], BF16, tag="wvt")
                    nc.sync.dma_start(wvt[:], wv[nch])
                    t1vc = spw.tile([1, 2, 288], BF16, tag="t1vc")
                    nc.sync.dma_start(
                        t1vc[:],
                        tdram[w0:w0 + 2, 3072 + 288 * nch:3072 + 288 * (nch + 1)]
                        .unsqueeze(0))
                    for tch in range(4):       # token chunks of the pair
                        wh, ms = divmod(tch, 2)
                        vs = accp.tile([P, W2N], F32, tag="acc")
                        tsl = slice(tch * P, (tch + 1) * P)
                        for k in range(KC):
                            nc.tensor.matmul(vs[:, :288], hw[:, k, tsl],
                                             wvt[:, k, :],
                                             start=(k == 0), stop=False)
                        nc.tensor.matmul(
                            vs[:, :288], ones_b[:1, :P], t1vc[:1, wh, :],
                            start=False, stop=True)
                        nc.scalar.activation(
                            vsl[:, wh, ms, 4 * nch:4 * nch + 4, 1:73],
                            vs[:, :288].rearrange("p (h d) -> p h d", d=72),
                            AF.Copy)
                # attention, head-outer so expb loads once per pair
                ost = sp1.tile([P, H, W2N], BF16, tag="ost")
                nc.gpsimd.memset(ost[64:, :, :], 0.0)
                for h in range(H):
                    ebt = sps.tile([P, 2, N], BF16, tag="ebt")
                    nc.sync.dma_start(ebt[:], expb[h])
                    pieces = _qk_pieces(h)
                    for wh in range(2):
                        nsl = slice(wh * N, (wh + 1) * N)
                        pt = sps.tile([P, 2, N], BF16, tag="pt")
                        po = accp.tile([P, W2N], F32, tag="acc")
                        for ms in range(2):
                            ssp = accp.tile([P, W2N], F32, tag="acc")
                            msl = slice(wh * N + ms * P, wh * N + (ms + 1) * P)
                            for i, (sub, base, ln) in enumerate(pieces):
                                nc.tensor.matmul(
                                    ssp[:, :N],
                                    qkst[base:base + ln, KOFF + sub, msl],
                                    qkst[base:base + ln, sub, nsl],
                                    start=(i == 0),
                                    stop=(i == len(pieces) - 1),
                                    tile_position=(base, 0))
                            nc.scalar.activation(pt[:, ms, :], ssp[:, :N],
                                                 AF.Exp, scale=SCALE)
                            nc.vector.tensor_mul(pt[:, ms, :], pt[:, ms, :],
                                                 ebt[:, ms, :])
                        for ms in range(2):
                            nc.tensor.matmul(po[:73, :N],
                                             vsl[:, wh, ms, h, :],
                                             pt[:, ms, :],
                                             start=(ms == 0), stop=(ms == 1))
                        linv = spr.tile([1, N], F32, tag="linv")
                        nc.vector.reciprocal_approx_fast(linv[:], po[0:1, :N])
                        pbs = spr.tile([P, N], F32, tag="pbs")
                        nc.gpsimd.partition_broadcast(pbs[:73, :], linv[:],
                                                      channels=73)
                        nc.scalar.activation(ost[:73, h, nsl], po[:73, :N],
                                             AF.Copy)
                        nc.vector.tensor_mul(ost[:73, h, nsl],
                                             ost[:73, h, nsl], pbs[:73, :])
                # proj + residual -> xpd (fp32) + xpdb (bf16 shadow)
                for pc in range(KC):
                    wpt = spw.tile([P, H, P], BF16, tag="wpt")
                    nc.sync.dma_start(wpt[:], wps[pc])
                    yps = accp.tile([P, W2N], F32, tag="acc")
                    for h in range(H):
                        nc.tensor.matmul(yps[:], wpt[:, h, :], ost[:, h, :],
                                         start=(h == 0), stop=False)
                    nc.tensor.matmul(yps[:], bias2[:1, P * pc:P * (pc + 1)],
                                     ones_b[:1, :W2N], start=False, stop=True)
                    xres = spw.tile([P, 2, N], F32, tag="xres")
                    for wh in range(2):
                        nc.sync.dma_start(xres[:, wh, :],
                                          xT[w0 + wh, :, pc, :])
                    nc.vector.tensor_add(
                        xres[:], xres[:],
                        yps[:].rearrange("p (u n) -> p u n", n=N))
                    xrb = spw.tile([P, 2, N], BF16, tag="xrb")
                    nc.scalar.activation(
                        xrb[:].rearrange("p u n -> p (u n)"),
                        xres[:].rearrange("p u n -> p (u n)"), AF.Copy)
                    for wh in range(2):
                        nc.sync.dma_start(xpd[w0 + wh, :, pc, :],
                                          xres[:, wh, :])
                        nc.sync.dma_start(xpdb[w0 + wh, :, pc, :],
                                          xrb[:, wh, :])
                # LN2 from the bf16 shadow
                f_xp = fetch_dram_bf16(xpdb, w0, spw, "xpbs")
                bc2, bb2 = ln_pair(spw, rows, accp, f_xp, "ln2")
                hp = sp.tile([P, KC, W2N], BF16, tag="hw")
                for s in range(KC):
                    xbs = f_xp(s)
                    nc.vector.tensor_mul(hp[:, s, :], xbs[:], bc2[:])
                    nc.vector.tensor_add(hp[:, s, :], hp[:, s, :], bb2[:])
                # fc1 -> gelu -> h2a
                h2a = sp1.tile([P, M1T, W2N], BF16, tag="h2a")
                for m1 in range(M1T):
                    w1t = spw.tile([P, KC, P], BF16, tag="w1t")
                    nc.sync.dma_start(w1t[:], w1c[m1])
                    ps1 = accp.tile([P, W2N], F32, tag="acc")
                    for k in range(KC):
                        nc.tensor.matmul(ps1[:], w1t[:, k, :], hp[:, k, :],
                                         start=(k == 0), stop=(k == KC - 1))
                    h2c = h2a[:, m1, :]
                    if not sim_gelu:
                        nc.scalar.activation(h2c, ps1[:], AF.Gelu_apprx_tanh,
                                             bias=f1bs[:, m1:m1 + 1])
                    else:
                        u = rows.tile([P, W2N], F32, tag="gelu_u")
                        nc.vector.tensor_add(
                            u[:], ps1[:],
                            f1bs[:, m1:m1 + 1].to_broadcast((P, W2N)))
                        t3 = rows.tile([P, W2N], F32, tag="gelu_t3")
                        nc.vector.tensor_mul(t3[:], u[:], u[:])
                        nc.vector.tensor_mul(t3[:], t3[:], u[:])
                        nc.vector.scalar_tensor_tensor(
                            t3[:], t3[:], 0.044715, u[:],
                            mybir.AluOpType.mult, mybir.AluOpType.add)
                        nc.scalar.activation(t3[:], t3[:], AF.Tanh,
                                             scale=0.7978845608028654)
                        nc.vector.scalar_tensor_tensor(
                            t3[:], t3[:], 1.0, u[:],
                            mybir.AluOpType.add, mybir.AluOpType.mult)
                        nc.vector.tensor_scalar_mul(h2c, t3[:], 0.5)
                # fc2 + residual + output
                for pm in range(KC):
                    w2t = spw.tile([P, M1T, P], BF16, tag="w2t")
                    nc.sync.dma_start(w2t[:], w2[pm])
                    ps2 = accp.tile([P, W2N], F32, tag="acc")
                    for m1 in range(M1T):
                        nc.tensor.matmul(ps2[:], w2t[:, m1, :], h2a[:, m1, :],
                                         start=(m1 == 0), stop=False)
                    nc.tensor.matmul(
                        ps2[:], bias2[:1, C + P * pm:C + P * (pm + 1)],
                        ones_b[:1, :W2N], start=False, stop=True)
                    xps = spw.tile([P, 2, N], F32, tag="xps")
                    for wh in range(2):
                        nc.sync.dma_start(xps[:, wh, :],
                                          xpd[w0 + wh, :, pm, :])
                    ot = spw.tile([P, 2, N], F32, tag="ot")
                    nc.vector.tensor_add(
                        ot[:], xps[:],
                        ps2[:].rearrange("p (u n) -> p u n", n=N))
                    for wh in range(2):
                        nc.sync.dma_start(outT[w0 + wh, :, pm, :],
                                          ot[:, wh, :])

    nc.compile()
    return nc


# ---------------------------------------------------------------------------
# host side
# ---------------------------------------------------------------------------

def _qk_colmap():
    m = np.full(2 * H * HS, -1, np.int64)
    for h in range(H):
        m[HS * h:HS * h + DH] = np.arange(72 * h, 72 * h + 72)
        m[H * HS + HS * h:H * HS + HS * h + DH] = \
            np.arange(C + 72 * h, C + 72 * h + 72)
    return m


def _prep_core_inputs(x_c, c_c, wdict):
    """x_c: [nw, N, C], c_c: [nw, C] -> per-core input map"""
    nw = x_c.shape[0]
    xT = np.ascontiguousarray(
        x_c.transpose(0, 2, 1).reshape(nw, KC, P, N).transpose(
            0, 2, 1, 3)).astype(np.float32)
    caug = np.zeros((nw, 1280), np.float32)
    caug[:, :C] = c_c
    caug[:, C] = 1.0
    cT = np.ascontiguousarray(caug.T.reshape(10, P, nw)).astype(NPBF16)
    return {"xT": xT, "xTb": xT.astype(NPBF16), "cT": cT, **wdict}


def _prep_weights(qkv_w, qkv_b, qkvt_w, qkvt_b, rpb_table, rel_idx,
                  proj_w, proj_b, fc1_w, fc1_b, fc2_w, fc2_b):
    qkmap = _qk_colmap()
    amap = np.concatenate([qkmap, np.arange(2 * C, 3 * C)])  # 4224 cols
    valid = amap >= 0

    wct = np.zeros((1280, 4224), np.float32)
    wct[:C, valid] = qkvt_w[amap[valid], :].T
    wct[C, valid] = (qkv_b + qkvt_b)[amap[valid]]
    wct = wct.reshape(10, P, 4224).astype(NPBF16)

    nqk = 2 * H * HS
    wqkT = np.zeros((C, nqk), np.float32)
    wqkT[:, valid[:nqk]] = qkv_w[qkmap[valid[:nqk]], :].T
    wqk = np.ascontiguousarray(
        wqkT.reshape(KC, P, QKM, P).transpose(2, 1, 0, 3)).astype(NPBF16)

    wv = np.ascontiguousarray(
        qkv_w[2 * C:, :].T.reshape(KC, P, 4, 288).transpose(
            2, 1, 0, 3)).astype(NPBF16)

    bias = rpb_table[rel_idx]                      # [N(n), N(m), H]
    expb = np.ascontiguousarray(
        np.exp(bias).transpose(2, 1, 0).reshape(H, 2, P, N).transpose(
            0, 2, 1, 3)).astype(NPBF16)

    wp_sl = np.zeros((P, H, C), np.float32)        # [slot-row d, head, p]
    for h in range(H):
        wp_sl[1:73, h, :] = proj_w[:, 72 * h:72 * h + 72].T
    wps = np.ascontiguousarray(
        wp_sl.reshape(P, H, KC, P).transpose(2, 0, 1, 3)).astype(NPBF16)

    w1c = np.ascontiguousarray(
        fc1_w.T.reshape(KC, P, M1T, P).transpose(2, 1, 0, 3)).astype(NPBF16)
    w2 = np.ascontiguousarray(
        fc2_w.T.reshape(M1T, P, KC, P).transpose(2, 1, 0, 3)).astype(NPBF16)
    f1b = np.ascontiguousarray(fc1_b.reshape(M1T, P).T).astype(np.float32)
    b2 = np.concatenate([proj_b, fc2_b]).reshape(1, 2 * C).astype(NPBF16)

    return {"wct": wct, "wqk": wqk, "wv": wv, "expb": expb, "wps": wps,
            "w1c": w1c, "w2": w2, "f1b": f1b, "b2": b2}


_PROGRAM = None


def kernel(x, c, qkv_w, qkv_b, qkvt_w, qkvt_b, rpb_table, proj_w, proj_b,
           fc1_w, fc1_b, fc2_w, fc2_b, rel_idx, _trace=False):
    global _PROGRAM
    x = np.asarray(x, np.float32)
    c = np.asarray(c, np.float32)
    wdict = _prep_weights(
        np.asarray(qkv_w, np.float32), np.asarray(qkv_b, np.float32),
        np.asarray(qkvt_w, np.float32), np.asarray(qkvt_b, np.float32),
        np.asarray(rpb_table, np.float32), np.asarray(rel_idx),
        np.asarray(proj_w, np.float32), np.asarray(proj_b, np.float32),
        np.asarray(fc1_w, np.float32), np.asarray(fc1_b, np.float32),
        np.asarray(fc2_w, np.float32), np.asarray(fc2_b, np.float32))

    if _PROGRAM is None:
        _PROGRAM = build_program(NW)
    nc = _PROGRAM

    in_maps = []
    for core in range(NCORES):
        sl = slice(core * NW, (core + 1) * NW)
        in_maps.append(_prep_core_inputs(x[sl], c[sl], wdict))

    res = bass_utils.run_bass_kernel_spmd(
        nc, in_maps, core_ids=list(range(NCORES)), trace=_trace)

    out = np.empty((B, N, C), np.float32)
    for core in range(NCORES):
        oT = res.results[core]["outT"]            # [NW, P, KC, N]
        out[core * NW:(core + 1) * NW] = \
            oT.transpose(0, 2, 1, 3).reshape(NW, C, N).transpose(0, 2, 1)
    if _trace:
        return out, res
    return out



# revision 8
# speedup vs baseline: 1.0062x; 1.0062x over previous
"""DiffiT transformer block kernel for 8 Trainium2 NeuronCores.

Data-parallel over the B=64 window axis (8 windows per core). Activations
are feature-major ([channel, token]) so every linear contracts over the
SBUF partition axis. Q/K stay feature-major with heads packed at a 96-row
stride (so each head's 72 rows sit at 32-aligned partition bases and the
per-head score matmuls can slice them legally); V is produced token-major
into per-head slots with an appended ones-column, so O^T = V_aug.T @ P^T
yields the softmax denominator as row 72. Per-token scalars (LN mean/rstd,
softmax 1/l) are broadcast across partitions with K=1 ones-matmuls on the
PE. Dense matmuls run bf16; the residual stream stays fp32; small fixup
matmuls use float32r (full-rate fp32 at free-dim >= 256).

All biases and the time-token conditioning (c @ qkvt^T + biases) enter as
rank-1 (K=1) matmul fixups folded into the PSUM accumulations.
"""

import math
from contextlib import ExitStack

import numpy as np
import ml_dtypes

import concourse.bass as bass
import concourse.mybir as mybir
import concourse.tile as tile
from concourse import bacc
from concourse import bass_utils

F32 = mybir.dt.float32
F32R = mybir.dt.float32r
BF16 = mybir.dt.bfloat16
NPBF16 = ml_dtypes.bfloat16
AF = mybir.ActivationFunctionType

P = 128
WS = 16
N = 256            # tokens per window
C = 1152           # hidden
H = 16             # heads
DH = 72            # head dim
HS = 96            # head stride in the QK packing (32-aligned, >= DH)
MLP = 4608
EPS = 1e-6
B = 64
NCORES = 8
NW = B // NCORES   # windows per core
KC = C // P        # 9  k-tiles over the hidden dim
QKM = 2 * H * HS // P   # 24 m-tiles over packed Q+K (96-stride)
KOFF = QKM // 2    # first K-side m-tile
M1T = MLP // P     # 36 fc1 row tiles
SCALE = 1.0 / math.sqrt(DH)


def _r(ap):
    """view a 4-byte fp32 AP as float32r for full-rate PE matmuls"""
    return ap.bitcast(F32R)


def _qk_pieces(h):
    """32-aligned partition pieces covering head h's 72 rows in the
    96-stride packing: [(subtile, base, length), ...]; piece legality:
    base 0 any len, base 64 len<=64, base 32/96 len<=32."""
    start, end = HS * h, HS * h + DH
    out = []
    while start < end:
        sub, base = divmod(start, P)
        ln = min(end - start, P - base)
        if base == 64:
            ln = min(ln, 64)
        elif base in (32, 96):
            ln = min(ln, 32)
        elif base != 0:
            raise AssertionError(base)
        out.append((sub, base, ln))
        start += ln
    return out


def build_program(nw=NW, sim_gelu=False):
    nc = bacc.Bacc("TRN2", target_bir_lowering=False, debug=False,
                   num_devices=NCORES)

    # register the layernorm epsilon as a const AP (activation float biases
    # other than 0.0/1.0 need one), same pattern as Bass.__init__
    eps_t = nc.alloc_sbuf_tensor("const-eps", [P, 1], F32)
    nc.gpsimd.memset(eps_t.ap(), EPS)
    nc.const_aps.aps[(F32, EPS)] = eps_t.ap()
    nc.all_engine_barrier()

    def din(name, shape, dt):
        return nc.dram_tensor(name, shape, dt, kind="ExternalInput").ap()

    xT = din("xT", [nw, P, KC, N], F32)          # x, feature-major
    xTb = din("xTb", [nw, P, KC, N], BF16)       # x, bf16 copy for LN stats
    cT = din("cT", [10, P, nw], BF16)            # c augmented with ones row
    wct = din("wct", [10, P, 4224], BF16)        # qkvt^T reordered + bias row
    wqk = din("wqk", [QKM, P, KC, P], BF16)      # qkv^T QK part, 96-stride
    wv = din("wv", [4, P, KC, 288], BF16)        # qkv^T V part, chunk-major
    expb = din("expb", [H, P, 2, N], BF16)       # exp(rel-pos bias)^T per head
    wps = din("wps", [KC, P, H, P], BF16)        # proj^T, head-slot padded
    w1c = din("w1c", [M1T, P, KC, P], BF16)      # fc1^T pre-chunked
    w2 = din("w2", [KC, P, M1T, P], BF16)        # fc2^T, pm-chunked
    f1b = din("f1b", [P, M1T], F32)              # fc1 bias, per-partition
    b2 = din("b2", [1, 2 * C], BF16)             # proj_b ++ fc2_b
    outT = nc.dram_tensor("outT", [nw, P, KC, N], F32,
                          kind="ExternalOutput").ap()

    NPAIR = nw // 2
    W2N = 2 * N        # tokens per window pair

    with tile.TileContext(nc) as tc, ExitStack() as ctx:
        keep = ctx.enter_context(tc.tile_pool(name="keep", bufs=1))
        dram = ctx.enter_context(tc.tile_pool(name="dram", bufs=1,
                                              space="DRAM"))

        ones_b = keep.tile([1, W2N], BF16, tag="ones_b")  # bf16 rhs of K=1
        ones_c = keep.tile([P, 1], BF16, tag="ones_c")    # lhsT of column sums
        nc.gpsimd.memset(ones_b[:], 1.0)
        nc.gpsimd.memset(ones_c[:], 1.0)
        bias2 = keep.tile([1, 2 * C], BF16, tag="bias2")
        nc.sync.dma_start(bias2[:], b2[:])
        f1bs = keep.tile([P, M1T], F32, tag="f1bs")
        nc.sync.dma_start(f1bs[:], f1b[:])

        tdram = dram.tile([nw, 4224], BF16)
        xpd = dram.tile([nw, P, KC, N], F32)     # x after attention branch
        xpdb = dram.tile([nw, P, KC, N], BF16)   # bf16 shadow for LN2

        # ---- phase 0: conditioning T = c_aug @ W_ct ----------------------
        with tc.tile_pool(name="ph0", bufs=2) as p0, \
             tc.tile_pool(name="ph0p", bufs=2, space="PSUM") as pp0:
            caug = p0.tile([P, 10, nw], BF16, tag="caug")
            nc.sync.dma_start(caug[:], cT.rearrange("k p w -> p k w"))
            tsb = p0.tile([8, 4224], BF16, tag="tsb")
            for i in range(9):
                n0, nl = i * 512, min(512, 4224 - i * 512)
                tps = pp0.tile([8, 512], F32, tag="tps")
                for k in range(10):
                    wt = p0.tile([P, 512], BF16, tag="wctt")
                    nc.sync.dma_start(wt[:, :nl], wct[k, :, n0:n0 + nl])
                    nc.tensor.matmul(tps[:nw, :nl], caug[:, k, :], wt[:, :nl],
                                     start=(k == 0), stop=(k == 9))
                nc.scalar.activation(tsb[:nw, n0:n0 + nl], tps[:nw, :nl],
                                     AF.Copy)
            nc.sync.dma_start(tdram[:, :], tsb[:nw, :])

        # ---- layernorm for a window pair -> PSUM broadcast [P, W2N] ------
        # acc-tile layout: [:, :N]+[:, N:] hold the two windows; returns one
        # [P, W2N] psum tile pair (rstd bcast, -mean*rstd bcast)
        def ln_pair(pool, rows, accp, fetch, tag):
            """fetch(s) -> [P, W2N] bf16 tile of the LN input, sub-tile s.
            Returns (rstd_bcast, -mean*rstd bcast) PSUM tiles."""
            ms0 = accp.tile([1, W2N], F32, tag="acc")
            ms1 = accp.tile([1, W2N], F32, tag="acc")
            for s in range(KC):
                xbs = fetch(s)
                xsq = pool.tile([P, W2N], BF16, tag=tag + "xsq")
                nc.vector.tensor_mul(xsq[:], xbs[:], xbs[:])
                nc.tensor.matmul(ms0[:], ones_c[:], xbs[:],
                                 start=(s == 0), stop=(s == KC - 1))
                nc.tensor.matmul(ms1[:], ones_c[:], xsq[:],
                                 start=(s == 0), stop=(s == KC - 1))
            mean = rows.tile([1, W2N], F32, tag="r_mean")
            ra = rows.tile([1, W2N], F32, tag="r_a")
            rb = rows.tile([1, W2N], F32, tag="r_b")
            nc.vector.tensor_scalar_mul(mean[:], ms0[:], 1.0 / C)
            nc.vector.tensor_scalar_mul(ra[:], ms1[:], 1.0 / C)   # E[x^2]
            nc.vector.tensor_mul(rb[:], mean[:], mean[:])         # mean^2
            nc.vector.tensor_sub(ra[:], ra[:], rb[:])             # var
            nc.scalar.activation(rb[:], ra[:], AF.Sqrt, bias=EPS) # sd
            nc.vector.reciprocal_approx_fast(ra[:], rb[:])        # 1/sd
            rstd = rows.tile([1, W2N], BF16, tag="r_rstd")
            nc.gpsimd.tensor_copy(rstd[:], ra[:])
            bneg = rows.tile([1, W2N], BF16, tag="r_bneg")
            nc.vector.scalar_tensor_tensor(
                bneg[:], mean[:], -1.0, rstd[:],
                mybir.AluOpType.mult, mybir.AluOpType.mult)
            bc = accp.tile([P, W2N], F32, tag="acc")
            nc.tensor.matmul(bc[:], ones_b[:1, :P], rstd[:],
                             start=True, stop=True)
            bb = accp.tile([P, W2N], F32, tag="acc")
            nc.tensor.matmul(bb[:], ones_b[:1, :P], bneg[:],
                             start=True, stop=True)
            return bc, bb

        # ==== attention superphase: per pair LN1 -> QKV -> attn -> proj ===
        with tc.tile_pool(name="sp", bufs=2) as sp, \
             tc.tile_pool(name="sp1", bufs=1) as sp1, \
             tc.tile_pool(name="spw", bufs=2) as spw, \
             tc.tile_pool(name="sps", bufs=3) as sps, \
             tc.tile_pool(name="spr", bufs=2) as spr, \
             tc.tile_pool(name="rows", bufs=1) as rows, \
             tc.tile_pool(name="accp", bufs=8, space="PSUM") as accp:
            def fetch_dram_bf16(src, w0, pool, tag):
                def fetch(s):
                    t = pool.tile([P, W2N], BF16, tag=tag)
                    for wh in range(2):
                        nc.sync.dma_start(t[:, wh * N:(wh + 1) * N],
                                          src[w0 + wh, :, s, :])
                    return t
                return fetch

            for pr in range(NPAIR):
                w0 = 2 * pr
                ctx2 = ExitStack()
                ctx2.enter_context(nc.named_scope(f"p{pr}_ln1"))
                f_x = fetch_dram_bf16(xTb, w0, spw, "xbs")
                bc, bb = ln_pair(spw, rows, accp, f_x, "ln1")
                hw = sp.tile([P, KC, W2N], BF16, tag="hw")
                for s in range(KC):
                    xbs = f_x(s)
                    nc.vector.tensor_mul(hw[:, s, :], xbs[:], bc[:])
                    nc.vector.tensor_add(hw[:, s, :], hw[:, s, :], bb[:])
                ctx2.close()
                ctx2 = ExitStack()
                ctx2.enter_context(nc.named_scope(f"p{pr}_qk"))
                # QK (96-stride packed), N = both windows
                qkst = sp1.tile([P, QKM, W2N], BF16, tag="qkst")
                for m in range(QKM):
                    wt = spw.tile([P, KC, P], BF16, tag="wqkt")
                    nc.sync.dma_start(wt[:], wqk[m])
                    t1m = spw.tile([1, 2, P], BF16, tag="t1m")
                    nc.sync.dma_start(
                        t1m[:], tdram[w0:w0 + 2, P * m:P * (m + 1)]
                        .unsqueeze(0))
                    qs = accp.tile([P, W2N], F32, tag="acc")
                    for k in range(KC):
                        nc.tensor.matmul(qs[:], wt[:, k, :], hw[:, k, :],
                                         start=(k == 0), stop=False)
                    nc.tensor.matmul(qs[:, :N], t1m[:1, 0, :],
                                     ones_b[:1, :N], start=False, stop=False)
                    nc.tensor.matmul(qs[:, N:], t1m[:1, 1, :],
                                     ones_b[:1, :N], start=False, stop=True)
                    nc.scalar.activation(qkst[:, m, :], qs[:], AF.Copy)
                ctx2.close()
                ctx2 = ExitStack()
                ctx2.enter_context(nc.named_scope(f"p{pr}_v"))
                # V token-major into per-head slots (ones in col 0)
                vsl = sp1.tile([P, 2, 2, H, 73], BF16, tag="vsl")
                nc.vector.memset(vsl[:, :, :, :, 0:1], 1.0)
                for nch in range(4):
                    wvt = spw.tile([P, KC, 288], BF16, tag="wvt")
                    nc.sync.dma_start(wvt[:], wv[nch])
                    t1vc = spw.tile([1, 2, 288], BF16, tag="t1vc")
                    nc.sync.dma_start(
                        t1vc[:],
                        tdram[w0:w0 + 2, 3072 + 288 * nch:3072 + 288 * (nch + 1)]
                        .unsqueeze(0))
                    for tch in range(4):       # token chunks of the pair
                        wh, ms = divmod(tch, 2)
                        vs = accp.tile([P, W2N], F32, tag="acc")
                        tsl = slice(tch * P, (tch + 1) * P)
                        for k in range(KC):
                            nc.tensor.matmul(vs[:, :288], hw[:, k, tsl],
                                             wvt[:, k, :],
                                             start=(k == 0), stop=False)
                        nc.tensor.matmul(
                            vs[:, :288], ones_b[:1, :P], t1vc[:1, wh, :],
                            start=False, stop=True)
                        nc.scalar.activation(
                            vsl[:, wh, ms, 4 * nch:4 * nch + 4, 1:73],
                            vs[:, :288].rearrange("p (h d) -> p h d", d=72),
                            AF.Copy)
                ctx2.close()
                ctx2 = ExitStack()
                ctx2.enter_context(nc.named_scope(f"p{pr}_attn"))
                # attention, head-outer so expb loads once per pair
                ost = sp1.tile([P, H, W2N], BF16, tag="ost")
                nc.gpsimd.memset(ost[64:, :, :], 0.0)
                for h in range(H):
                    ebt = sps.tile([P, 2, N], BF16, tag="ebt")
                    nc.sync.dma_start(ebt[:], expb[h])
                    pieces = _qk_pieces(h)
                    for wh in range(2):
                        nsl = slice(wh * N, (wh + 1) * N)
                        pt = sps.tile([P, 2, N], BF16, tag="pt")
                        po = accp.tile([P, W2N], F32, tag="acc")
                        for ms in range(2):
                            ssp = accp.tile([P, W2N], F32, tag="acc")
                            msl = slice(wh * N + ms * P, wh * N + (ms + 1) * P)
                            for i, (sub, base, ln) in enumerate(pieces):
                                nc.tensor.matmul(
                                    ssp[:, :N],
                                    qkst[base:base + ln, KOFF + sub, msl],
                                    qkst[base:base + ln, sub, nsl],
                                    start=(i == 0),
                                    stop=(i == len(pieces) - 1),
                                    tile_position=(base, 0))
                            nc.scalar.activation(pt[:, ms, :], ssp[:, :N],
                                                 AF.Exp, scale=SCALE)
                            nc.vector.tensor_mul(pt[:, ms, :], pt[:, ms, :],
                                                 ebt[:, ms, :])
                        for ms in range(2):
                            nc.tensor.matmul(po[:73, :N],
                                             vsl[:, wh, ms, h, :],
                                             pt[:, ms, :],
                                             start=(ms == 0), stop=(ms == 1))
                        linv = spr.tile([1, N], F32, tag="linv")
                        nc.vector.reciprocal_approx_fast(linv[:], po[0:1, :N])
                        pbs = spr.tile([P, N], F32, tag="pbs")
                        nc.gpsimd.partition_broadcast(pbs[:73, :], linv[:],
                                                      channels=73)
                        nc.scalar.activation(ost[:73, h, nsl], po[:73, :N],
                                             AF.Copy)
                        nc.vector.tensor_mul(ost[:73, h, nsl],
                                             ost[:73, h, nsl], pbs[:73, :])
                ctx2.close()
                ctx2 = ExitStack()
                ctx2.enter_context(nc.named_scope(f"p{pr}_proj"))
                # proj + residual -> xpd (fp32) + xpdb (bf16 shadow)
                for pc in range(KC):
                    wpt = spw.tile([P, H, P], BF16, tag="wpt")
                    nc.sync.dma_start(wpt[:], wps[pc])
                    yps = accp.tile([P, W2N], F32, tag="acc")
                    for h in range(H):
                        nc.tensor.matmul(yps[:], wpt[:, h, :], ost[:, h, :],
                                         start=(h == 0), stop=False)
                    nc.tensor.matmul(yps[:], bias2[:1, P * pc:P * (pc + 1)],
                                     ones_b[:1, :W2N], start=False, stop=True)
                    xres = spw.tile([P, 2, N], F32, tag="xres")
                    for wh in range(2):
                        nc.sync.dma_start(xres[:, wh, :],
                                          xT[w0 + wh, :, pc, :])
                    nc.vector.tensor_add(
                        xres[:], xres[:],
                        yps[:].rearrange("p (u n) -> p u n", n=N))
                    xrb = spw.tile([P, 2, N], BF16, tag="xrb")
                    nc.scalar.activation(
                        xrb[:].rearrange("p u n -> p (u n)"),
                        xres[:].rearrange("p u n -> p (u n)"), AF.Copy)
                    for wh in range(2):
                        nc.sync.dma_start(xpd[w0 + wh, :, pc, :],
                                          xres[:, wh, :])
                        nc.sync.dma_start(xpdb[w0 + wh, :, pc, :],
                                          xrb[:, wh, :])
                ctx2.close()
                ctx2 = ExitStack()
                ctx2.enter_context(nc.named_scope(f"p{pr}_ln2"))
                # LN2 from the bf16 shadow
                f_xp = fetch_dram_bf16(xpdb, w0, spw, "xpbs")
                bc2, bb2 = ln_pair(spw, rows, accp, f_xp, "ln2")
                hp = sp.tile([P, KC, W2N], BF16, tag="hw")
                for s in range(KC):
                    xbs = f_xp(s)
                    nc.vector.tensor_mul(hp[:, s, :], xbs[:], bc2[:])
                    nc.vector.tensor_add(hp[:, s, :], hp[:, s, :], bb2[:])
                ctx2.close()
                ctx2 = ExitStack()
                ctx2.enter_context(nc.named_scope(f"p{pr}_fc1"))
                # fc1 -> gelu -> h2a
                h2a = sp1.tile([P, M1T, W2N], BF16, tag="h2a")
                for m1 in range(M1T):
                    w1t = spw.tile([P, KC, P], BF16, tag="w1t")
                    nc.sync.dma_start(w1t[:], w1c[m1])
                    ps1 = accp.tile([P, W2N], F32, tag="acc")
                    for k in range(KC):
                        nc.tensor.matmul(ps1[:], w1t[:, k, :], hp[:, k, :],
                                         start=(k == 0), stop=(k == KC - 1))
                    h2c = h2a[:, m1, :]
                    if not sim_gelu:
                        nc.scalar.activation(h2c, ps1[:], AF.Gelu_apprx_tanh,
                                             bias=f1bs[:, m1:m1 + 1])
                    else:
                        u = rows.tile([P, W2N], F32, tag="gelu_u")
                        nc.vector.tensor_add(
                            u[:], ps1[:],
                            f1bs[:, m1:m1 + 1].to_broadcast((P, W2N)))
                        t3 = rows.tile([P, W2N], F32, tag="gelu_t3")
                        nc.vector.tensor_mul(t3[:], u[:], u[:])
                        nc.vector.tensor_mul(t3[:], t3[:], u[:])
                        nc.vector.scalar_tensor_tensor(
                            t3[:], t3[:], 0.044715, u[:],
                            mybir.AluOpType.mult, mybir.AluOpType.add)
                        nc.scalar.activation(t3[:], t3[:], AF.Tanh,
                                             scale=0.7978845608028654)
                        nc.vector.scalar_tensor_tensor(
                            t3[:], t3[:], 1.0, u[:],
                            mybir.AluOpType.add, mybir.AluOpType.mult)
                        nc.vector.tensor_scalar_mul(h2c, t3[:], 0.5)
                ctx2.close()
                ctx2 = ExitStack()
                ctx2.enter_context(nc.named_scope(f"p{pr}_fc2"))
                # fc2 + residual + output
                for pm in range(KC):
                    w2t = spw.tile([P, M1T, P], BF16, tag="w2t")
                    nc.sync.dma_start(w2t[:], w2[pm])
                    ps2 = accp.tile([P, W2N], F32, tag="acc")
                    for m1 in range(M1T):
                        nc.tensor.matmul(ps2[:], w2t[:, m1, :], h2a[:, m1, :],
                                         start=(m1 == 0), stop=False)
                    nc.tensor.matmul(
                        ps2[:], bias2[:1, C + P * pm:C + P * (pm + 1)],
                        ones_b[:1, :W2N], start=False, stop=True)
                    xps = spw.tile([P, 2, N], F32, tag="xps")
                    for wh in range(2):
                        nc.sync.dma_start(xps[:, wh, :],
                                          xpd[w0 + wh, :, pm, :])
                    ot = spw.tile([P, 2, N], F32, tag="ot")
                    nc.vector.tensor_add(
                        ot[:], xps[:],
                        ps2[:].rearrange("p (u n) -> p u n", n=N))
                    for wh in range(2):
                        nc.sync.dma_start(outT[w0 + wh, :, pm, :],
                                          ot[:, wh, :])
                ctx2.close()

    nc.compile()
    return nc


# ---------------------------------------------------------------------------
# host side
# ---------------------------------------------------------------------------

def _qk_colmap():
    m = np.full(2 * H * HS, -1, np.int64)
    for h in range(H):
        m[HS * h:HS * h + DH] = np.arange(72 * h, 72 * h + 72)
        m[H * HS + HS * h:H * HS + HS * h + DH] = \
            np.arange(C + 72 * h, C + 72 * h + 72)
    return m


def _prep_core_inputs(x_c, c_c, wdict):
    """x_c: [nw, N, C], c_c: [nw, C] -> per-core input map"""
    nw = x_c.shape[0]
    xT = np.ascontiguousarray(
        x_c.transpose(0, 2, 1).reshape(nw, KC, P, N).transpose(
            0, 2, 1, 3)).astype(np.float32)
    caug = np.zeros((nw, 1280), np.float32)
    caug[:, :C] = c_c
    caug[:, C] = 1.0
    cT = np.ascontiguousarray(caug.T.reshape(10, P, nw)).astype(NPBF16)
    return {"xT": xT, "xTb": xT.astype(NPBF16), "cT": cT, **wdict}


def _prep_weights(qkv_w, qkv_b, qkvt_w, qkvt_b, rpb_table, rel_idx,
                  proj_w, proj_b, fc1_w, fc1_b, fc2_w, fc2_b):
    qkmap = _qk_colmap()
    amap = np.concatenate([qkmap, np.arange(2 * C, 3 * C)])  # 4224 cols
    valid = amap >= 0

    wct = np.zeros((1280, 4224), np.float32)
    wct[:C, valid] = qkvt_w[amap[valid], :].T
    wct[C, valid] = (qkv_b + qkvt_b)[amap[valid]]
    wct = wct.reshape(10, P, 4224).astype(NPBF16)

    nqk = 2 * H * HS
    wqkT = np.zeros((C, nqk), np.float32)
    wqkT[:, valid[:nqk]] = qkv_w[qkmap[valid[:nqk]], :].T
    wqk = np.ascontiguousarray(
        wqkT.reshape(KC, P, QKM, P).transpose(2, 1, 0, 3)).astype(NPBF16)

    wv = np.ascontiguousarray(
        qkv_w[2 * C:, :].T.reshape(KC, P, 4, 288).transpose(
            2, 1, 0, 3)).astype(NPBF16)

    bias = rpb_table[rel_idx]                      # [N(n), N(m), H]
    expb = np.ascontiguousarray(
        np.exp(bias).transpose(2, 1, 0).reshape(H, 2, P, N).transpose(
            0, 2, 1, 3)).astype(NPBF16)

    wp_sl = np.zeros((P, H, C), np.float32)        # [slot-row d, head, p]
    for h in range(H):
        wp_sl[1:73, h, :] = proj_w[:, 72 * h:72 * h + 72].T
    wps = np.ascontiguousarray(
        wp_sl.reshape(P, H, KC, P).transpose(2, 0, 1, 3)).astype(NPBF16)

    w1c = np.ascontiguousarray(
        fc1_w.T.reshape(KC, P, M1T, P).transpose(2, 1, 0, 3)).astype(NPBF16)
    w2 = np.ascontiguousarray(
        fc2_w.T.reshape(M1T, P, KC, P).transpose(2, 1, 0, 3)).astype(NPBF16)
    f1b = np.ascontiguousarray(fc1_b.reshape(M1T, P).T).astype(np.float32)
    b2 = np.concatenate([proj_b, fc2_b]).reshape(1, 2 * C).astype(NPBF16)

    return {"wct": wct, "wqk": wqk, "wv": wv, "expb": expb, "wps": wps,
            "w1c": w1c, "w2": w2, "f1b": f1b, "b2": b2}


_PROGRAM = None


def kernel(x, c, qkv_w, qkv_b, qkvt_w, qkvt_b, rpb_table, proj_w, proj_b,
           fc1_w, fc1_b, fc2_w, fc2_b, rel_idx, _trace=False):
    global _PROGRAM
    x = np.asarray(x, np.float32)
    c = np.asarray(c, np.float32)
    wdict = _prep_weights(
        np.asarray(qkv_w, np.float32), np.asarray(qkv_b, np.float32),
        np.asarray(qkvt_w, np.float32), np.asarray(qkvt_b, np.float32),
        np.asarray(rpb_table, np.float32), np.asarray(rel_idx),
        np.asarray(proj_w, np.float32), np.asarray(proj_b, np.float32),
        np.asarray(fc1_w, np.float32), np.asarray(fc1_b, np.float32),
        np.asarray(fc2_w, np.float32), np.asarray(fc2_b, np.float32))

    if _PROGRAM is None:
        _PROGRAM = build_program(NW)
    nc = _PROGRAM

    in_maps = []
    for core in range(NCORES):
        sl = slice(core * NW, (core + 1) * NW)
        in_maps.append(_prep_core_inputs(x[sl], c[sl], wdict))

    res = bass_utils.run_bass_kernel_spmd(
        nc, in_maps, core_ids=list(range(NCORES)), trace=_trace)

    out = np.empty((B, N, C), np.float32)
    for core in range(NCORES):
        oT = res.results[core]["outT"]            # [NW, P, KC, N]
        out[core * NW:(core + 1) * NW] = \
            oT.transpose(0, 2, 1, 3).reshape(NW, C, N).transpose(0, 2, 1)
    if _trace:
        return out, res
    return out



# revision 45
# speedup vs baseline: 1.2168x; 1.2093x over previous
"""DiffiT transformer block kernel for 8 Trainium2 NeuronCores.

Data-parallel over the B=64 window axis (8 windows per core). Activations
are feature-major ([channel, token]); every linear contracts over the SBUF
partition axis. Redesign v2 for tensor-engine density:

- x enters once as bf16 and stays resident in SBUF; the attention residual
  is kept on-chip (no DRAM roundtrip for the mid-residual).
- LN1 stats + broadcast tiles for ALL window pairs are computed in a
  prologue; each pair's input normalize (vector work) is emitted during the
  previous pair's fc1 so the PE never waits on LN at pair start.
- LN2 stat matmuls are interleaved into the proj loop with one-chunk lag.
- Softmax: per-head scores for both windows share one [128,512] PSUM tile,
  exp'd in one activation; the rel-pos bias multiply runs on gpsimd; the
  1/l normalize uses a PE ones-matmul broadcast + fused vector multiply
  (PSUM x PSUM -> SBUF bf16), emitted with one-head lookahead.
- O is packed tight feature-major, so proj contracts 9 (not 16) k-tiles.
- Rank-1 fixups (biases, time-token conditioning) fold into the PSUM
  accumulations as K=1/K=2 matmuls.
"""

import math
from contextlib import ExitStack

import numpy as np
import ml_dtypes

import concourse.bass as bass
import concourse.mybir as mybir
import concourse.tile as tile
from concourse import bacc
from concourse import bass_utils

F32 = mybir.dt.float32
BF16 = mybir.dt.bfloat16
NPBF16 = ml_dtypes.bfloat16
AF = mybir.ActivationFunctionType

P = 128
WS = 16
N = 256            # tokens per window
C = 1152           # hidden
H = 16             # heads
DH = 72            # head dim
HS = 96            # head stride in the QK packing (32-aligned, >= DH)
MLP = 4608
EPS = 1e-6
B = 64
NCORES = 8
NW = B // NCORES   # windows per core
KC = C // P        # 9  k-tiles over the hidden dim
QKM = 2 * H * HS // P   # 24 m-tiles over packed Q+K (96-stride)
KOFF = QKM // 2    # first K-side m-tile
M1T = MLP // P     # 36 fc1 row tiles
SCALE = 1.0 / math.sqrt(DH)
NPAIR = NW // 2
W2N = 2 * N        # tokens per window pair


def _qk_pieces(h):
    """32-aligned partition pieces covering head h's 72 rows in the
    96-stride packing: [(subtile, base, length), ...]; piece legality:
    base 0 any len, base 64 len<=64, base 32/96 len<=32."""
    start, end = HS * h, HS * h + DH
    out = []
    while start < end:
        sub, base = divmod(start, P)
        ln = min(end - start, P - base)
        if base == 64:
            ln = min(ln, 64)
        elif base in (32, 96):
            ln = min(ln, 32)
        elif base != 0:
            raise AssertionError(base)
        out.append((sub, base, ln))
        start += ln
    return out


def _ost_pieces(h):
    """pieces of the tight 72-row span of head h in the [P, KC, *] packing:
    [(subtile, partition base, src offset, length), ...]"""
    f0 = DH * h
    out = []
    off = 0
    while off < DH:
        sub, pb = divmod(f0 + off, P)
        ln = min(DH - off, P - pb)
        out.append((sub, pb, off, ln))
        off += ln
    return out


def build_program(nw=NW):
    nc = bacc.Bacc("TRN2", target_bir_lowering=False, debug=False,
                   num_devices=NCORES)

    # register the layernorm epsilon as a const AP (activation float biases
    # other than 0.0/1.0 need one), same pattern as Bass.__init__
    eps_t = nc.alloc_sbuf_tensor("const-eps", [P, 1], F32)
    nc.gpsimd.memset(eps_t.ap(), EPS)
    nc.const_aps.aps[(F32, EPS)] = eps_t.ap()
    nc.all_engine_barrier()

    def din(name, shape, dt):
        return nc.dram_tensor(name, shape, dt, kind="ExternalInput").ap()

    xb = din("xb", [nw, P, KC, N], BF16)         # x, feature-major bf16
    cT = din("cT", [10, P, nw], BF16)            # c augmented with ones row
    wct = din("wct", [10, P, 4224], BF16)        # qkvt^T reordered + bias row
    wqk = din("wqk", [QKM, P, KC, P], BF16)      # qkv^T QK part, 96-stride
    wv = din("wv", [4, P, KC, 288], BF16)        # qkv^T V part, chunk-major
    expb2 = din("expb2", [H, P, 2, W2N], BF16)   # exp(rel-pos bias)^T, 2x wide
    wps = din("wps", [KC, P, H, P], BF16)        # proj^T, head-slot padded
    w1c = din("w1c", [M1T, P, KC, P], BF16)      # fc1^T pre-chunked
    w2 = din("w2", [KC, P, M1T, P], BF16)        # fc2^T, pm-chunked
    f1b = din("f1b", [P, M1T], F32)              # fc1 bias, per-partition
    b2 = din("b2", [1, 2 * C], BF16)             # proj_b ++ fc2_b
    wpc = din("wpc", [2, W2N], BF16)             # window indicator rows
    outT = nc.dram_tensor("outT", [nw, P, KC, N], F32,
                          kind="ExternalOutput").ap()

    with tile.TileContext(nc) as tc, ExitStack() as ctx:
        keep = ctx.enter_context(tc.tile_pool(name="keep", bufs=1))
        dram = ctx.enter_context(tc.tile_pool(name="dram", bufs=1,
                                              space="DRAM"))

        ones_b = keep.tile([1, W2N], BF16, tag="ones_b")
        ones_c = keep.tile([P, 1], BF16, tag="ones_c")
        wpair = keep.tile([2, W2N], BF16, tag="wpair")   # window indicators
        nc.gpsimd.memset(ones_b[:], 1.0)
        nc.gpsimd.memset(ones_c[:], 1.0)
        nc.sync.dma_start(wpair[:], wpc[:])
        bias2 = keep.tile([1, 2 * C], BF16, tag="bias2")
        nc.sync.dma_start(bias2[:], b2[:])
        f1bs = keep.tile([P, M1T], F32, tag="f1bs")
        nc.sync.dma_start(f1bs[:], f1b[:])

        # resident activations / packings
        bcs = keep.tile([P, NPAIR, W2N], BF16, tag="bcs")    # LN1 rstd bcast
        bbs = keep.tile([P, NPAIR, W2N], BF16, tag="bbs")    # LN1 -m*r bcast
        qkst = keep.tile([P, QKM, W2N], BF16, tag="qkst")
        vsl = keep.tile([P, 2, 2, H, P], BF16, tag="vsl")    # padded V slots
        ost = keep.tile([P, H, W2N], BF16, tag="ost")        # O head slots
        xrb = keep.tile([P, KC, W2N], BF16, tag="xrb")       # x + attn (bf16)
        h2a = keep.tile([P, M1T, W2N], BF16, tag="h2a")
        bcs2 = keep.tile([P, W2N], BF16, tag="bcs2")         # LN2 bcasts
        bbs2 = keep.tile([P, W2N], BF16, tag="bbs2")

        tdram = dram.tile([nw, 4224], BF16)

        nc.vector.memset(vsl[:].rearrange("p a b h c -> p (a b h c)"), 0.0)
        nc.vector.memset(vsl[:, :, :, :, 0:1], 1.0)
        nc.vector.memset(ost[64:, :, :].rearrange("p h n -> p (h n)"), 0.0)

        # ---- phase 0: conditioning T = c_aug @ W_ct ----------------------
        with tc.tile_pool(name="ph0", bufs=2) as p0, \
             tc.tile_pool(name="ph0p", bufs=2, space="PSUM") as pp0:
            caug = p0.tile([P, 10, nw], BF16, tag="caug")
            nc.sync.dma_start(caug[:], cT.rearrange("k p w -> p k w"))
            tsb = p0.tile([8, 4224], BF16, tag="tsb")
            for i in range(9):
                n0, nl = i * 512, min(512, 4224 - i * 512)
                tps = pp0.tile([8, 512], F32, tag="tps")
                for k in range(10):
                    wt = p0.tile([P, 512], BF16, tag="wctt")
                    nc.sync.dma_start(wt[:, :nl], wct[k, :, n0:n0 + nl])
                    nc.tensor.matmul(tps[:nw, :nl], caug[:, k, :], wt[:, :nl],
                                     start=(k == 0), stop=(k == 9))
                nc.scalar.activation(tsb[:nw, n0:n0 + nl], tps[:nw, :nl],
                                     AF.Copy)
            nc.sync.dma_start(tdram[:, :], tsb[:nw, :])

        with tc.tile_pool(name="sp", bufs=2) as sp, \
             tc.tile_pool(name="spw", bufs=3) as spw, \
             tc.tile_pool(name="spv", bufs=2) as spv, \
             tc.tile_pool(name="spw2", bufs=2) as spw2, \
             tc.tile_pool(name="sps", bufs=2) as sps, \
             tc.tile_pool(name="spp", bufs=2) as spp, \
             tc.tile_pool(name="spr", bufs=2) as spr, \
             tc.tile_pool(name="spx", bufs=1) as spx, \
             tc.tile_pool(name="spt", bufs=1) as spt, \
             tc.tile_pool(name="rows", bufs=1) as rows, \
             tc.tile_pool(name="accp", bufs=6, space="PSUM") as accp, \
             tc.tile_pool(name="statp", bufs=1, space="PSUM") as statp:

            def x_slice(xp, s):
                return xp[:, s, :, :].rearrange("p u n -> p (u n)")

            def fetch_xchunk(pr, s):
                xch = sp.tile([P, 2, N], BF16, tag="xch")
                for wh in range(2):
                    nc.sync.dma_start(xch[:, wh, :], xb[2 * pr + wh, :, s, :])
                return xch[:].rearrange("p u n -> p (u n)")

            def load_xpair(pr):
                xp = spx.tile([P, KC, 2, N], BF16, tag="xp")
                for wh in range(2):
                    nc.sync.dma_start(xp[:, :, wh, :], xb[2 * pr + wh])
                return xp

            # LN stats for a pair from a fetch(s) -> [P, W2N] bf16 source.
            # Returns (rstd, bneg) bf16 [1, W2N] row tiles.
            def ln_rows(fetch, tag):
                ms0 = statp.tile([1, W2N], F32, tag="ms0")
                ms1 = statp.tile([1, W2N], F32, tag="ms1")
                for s in range(KC):
                    xbs = fetch(s)
                    xsq = sp.tile([P, W2N], BF16, tag="xsq")
                    nc.vector.tensor_mul(xsq[:], xbs, xbs)
                    nc.tensor.matmul(ms0[:], ones_c[:], xbs,
                                     start=(s == 0), stop=(s == KC - 1))
                    nc.tensor.matmul(ms1[:], ones_c[:], xsq[:],
                                     start=(s == 0), stop=(s == KC - 1))
                return ln_rows_tail(ms0, ms1, tag)

            def ln_rows_tail(ms0, ms1, tag):
                mean = rows.tile([1, W2N], F32, tag="r_mean")
                ra = rows.tile([1, W2N], F32, tag="r_a")
                rb = rows.tile([1, W2N], F32, tag="r_b")
                nc.vector.tensor_scalar_mul(mean[:], ms0[:], 1.0 / C)
                nc.vector.tensor_scalar_mul(ra[:], ms1[:], 1.0 / C)
                nc.vector.tensor_mul(rb[:], mean[:], mean[:])
                nc.vector.tensor_sub(ra[:], ra[:], rb[:])      # var
                nc.scalar.activation(rb[:], ra[:], AF.Sqrt, bias=EPS)
                nc.vector.reciprocal_approx_fast(ra[:], rb[:])
                rstd = rows.tile([1, W2N], BF16, tag="r_rstd")
                nc.vector.tensor_copy(rstd[:], ra[:])
                bneg = rows.tile([1, W2N], BF16, tag="r_bneg")
                nc.vector.scalar_tensor_tensor(
                    bneg[:], mean[:], -1.0, rstd[:],
                    mybir.AluOpType.mult, mybir.AluOpType.mult)
                return rstd, bneg

            def bcast_rows(rstd, bneg, dst_c, dst_b):
                bcp = accp.tile([P, W2N], F32, tag="acc")
                nc.tensor.matmul(bcp[:], ones_b[:1, :P], rstd[:],
                                 start=True, stop=True)
                bbp = accp.tile([P, W2N], F32, tag="acc")
                nc.tensor.matmul(bbp[:], ones_b[:1, :P], bneg[:],
                                 start=True, stop=True)
                nc.scalar.activation(dst_c, bcp[:], AF.Copy)
                nc.scalar.activation(dst_b, bbp[:], AF.Copy)

            # normalize: dst[:, s, :] = src(s) * bc + bb, alternating engines
            def normalize(dst, fetch, bc, bb):
                for s in range(KC):
                    eng = nc.vector if s % 2 == 0 else nc.gpsimd
                    eng.tensor_mul(dst[:, s, :], fetch(s), bc)
                    eng.tensor_add(dst[:, s, :], dst[:, s, :], bb)

            # ---- prologue: LN1 stats/broadcasts for all pairs ------------
            for pr in range(NPAIR):
                rstd, bneg = ln_rows(
                    lambda s, pr=pr: fetch_xchunk(pr, s), f"l1_{pr}")
                bcast_rows(rstd, bneg, bcs[:, pr, :], bbs[:, pr, :])

            xp = load_xpair(0)
            hw = sp.tile([P, KC, W2N], BF16, tag="hw")
            normalize(hw, lambda s: x_slice(xp, s),
                      bcs[:, 0, :], bbs[:, 0, :])

            # =================== main loop over pairs =====================
            for pr in range(NPAIR):
                w0 = 2 * pr
                # ---- QK (96-stride packed) -------------------------------
                t2all = spt.tile([2, QKM * P], BF16, tag="t2all")
                nc.sync.dma_start(t2all[:], tdram[w0:w0 + 2, :QKM * P])
                for m in range(QKM):
                    wt = spw.tile([P, KC, P], BF16, tag="wt")
                    nc.sync.dma_start(wt[:], wqk[m])
                    qs = accp.tile([P, W2N], F32, tag="acc")
                    for k in range(KC):
                        nc.tensor.matmul(qs[:], wt[:, k, :], hw[:, k, :],
                                         start=(k == 0), stop=False)
                    nc.tensor.matmul(qs[:], t2all[:, P * m:P * (m + 1)],
                                     wpair[:], start=False, stop=True)
                    nc.scalar.activation(qkst[:, m, :], qs[:], AF.Copy)
                # ---- V token-major into padded per-head slots ------------
                for nch in range(4):
                    wvt = spv.tile([P, KC, 288], BF16, tag="wvt")
                    nc.sync.dma_start(wvt[:], wv[nch])
                    t1vc = spr.tile([1, 2, 288], BF16, tag="t1v")
                    nc.sync.dma_start(
                        t1vc[:],
                        tdram[w0:w0 + 2, 3072 + 288 * nch:3072 + 288 * (nch + 1)]
                        .unsqueeze(0))
                    for tch in range(4):
                        wh, ms = divmod(tch, 2)
                        vs = accp.tile([P, W2N], F32, tag="acc")
                        tsl = slice(tch * P, (tch + 1) * P)
                        for k in range(KC):
                            nc.tensor.matmul(vs[:, :288], hw[:, k, tsl],
                                             wvt[:, k, :],
                                             start=(k == 0), stop=False)
                        nc.tensor.matmul(
                            vs[:, :288], ones_b[:1, :P], t1vc[:1, wh, :],
                            start=False, stop=True)
                        nc.scalar.activation(
                            vsl[:, wh, ms, 4 * nch:4 * nch + 4, 1:73],
                            vs[:, :288].rearrange("p (h d) -> p h d", d=72),
                            AF.Copy)
                # ---- attention, one-head lookahead normalize -------------
                def emit_norm(h, pt):
                    for wh in range(2):
                        polb = accp.tile([P, W2N], F32, tag="acc")
                        for ms in range(2):
                            nc.tensor.matmul(
                                polb[:, :N], vsl[:, wh, ms, h, :],
                                pt[:, ms, wh * N:(wh + 1) * N],
                                start=(ms == 0), stop=(ms == 1))
                        linv32 = spr.tile([1, N], F32, tag="linv32")
                        nc.vector.reciprocal_approx_fast(linv32[:],
                                                         polb[0:1, :N])
                        linv = spr.tile([1, N], BF16, tag="linv")
                        nc.vector.tensor_copy(linv[:], linv32[:])
                        nc.tensor.matmul(polb[:73, N:], ones_b[:1, :73],
                                         linv[:], start=True, stop=True)
                        obf = spr.tile([P, N], BF16, tag="obf")
                        nc.vector.tensor_copy(obf[:73, :], polb[:73, :N])
                        nc.vector.tensor_mul(
                            ost[:73, h, wh * N:(wh + 1) * N],
                            obf[:73, :], polb[:73, N:])

                prev = None
                for h in range(H):
                    et = sps.tile([P, 2, W2N], BF16, tag="ebt")
                    nc.sync.dma_start(et[:], expb2[h])
                    pt = spp.tile([P, 2, W2N], BF16, tag="pt")
                    qp = _qk_pieces(h)
                    for ms in range(2):
                        ssp = accp.tile([P, W2N], F32, tag="acc")
                        for wh in range(2):
                            msl = slice(wh * N + ms * P, wh * N + (ms + 1) * P)
                            nsl = slice(wh * N, (wh + 1) * N)
                            for i, (sub, base, ln) in enumerate(qp):
                                nc.tensor.matmul(
                                    ssp[:, nsl],
                                    qkst[base:base + ln, KOFF + sub, msl],
                                    qkst[base:base + ln, sub, nsl],
                                    start=(i == 0),
                                    stop=(i == len(qp) - 1),
                                    tile_position=(base, 0))
                        nc.scalar.activation(pt[:, ms, :], ssp[:], AF.Exp,
                                             scale=SCALE)
                    nc.gpsimd.tensor_mul(
                        pt[:].rearrange("p u n -> p (u n)"),
                        pt[:].rearrange("p u n -> p (u n)"),
                        et[:].rearrange("p u n -> p (u n)"))
                    if prev is not None:
                        emit_norm(*prev)
                    prev = (h, pt)
                emit_norm(*prev)
                # ---- proj + residual + interleaved LN2 stats -------------
                ms0 = statp.tile([1, W2N], F32, tag="ms0")
                ms1 = statp.tile([1, W2N], F32, tag="ms1")

                def emit_stats(s):
                    xsq = sp.tile([P, W2N], BF16, tag="xsq")
                    nc.vector.tensor_mul(xsq[:], xrb[:, s, :], xrb[:, s, :])
                    nc.tensor.matmul(ms0[:], ones_c[:], xrb[:, s, :],
                                     start=(s == 0), stop=(s == KC - 1))
                    nc.tensor.matmul(ms1[:], ones_c[:], xsq[:],
                                     start=(s == 0), stop=(s == KC - 1))

                HH = H // 2
                for pc in range(KC):
                    yps = accp.tile([P, W2N], F32, tag="acc")
                    for half in range(2):
                        wpt = spw2.tile([P, HH, P], BF16, tag="wpt")
                        nc.sync.dma_start(
                            wpt[:], wps[pc, :, HH * half:HH * (half + 1), :])
                        for hh in range(HH):
                            hg = HH * half + hh
                            nc.tensor.matmul(yps[:], wpt[:, hh, :],
                                             ost[:, hg, :],
                                             start=(hg == 0), stop=False)
                    nc.tensor.matmul(yps[:], bias2[:1, P * pc:P * (pc + 1)],
                                     ones_b[:1, :W2N], start=False, stop=True)
                    nc.vector.tensor_add(xrb[:, pc, :], x_slice(xp, pc),
                                         yps[:])
                    if pc > 0:
                        emit_stats(pc - 1)
                emit_stats(KC - 1)
                rstd2, bneg2 = ln_rows_tail(ms0, ms1, "l2")
                bcast_rows(rstd2, bneg2, bcs2[:], bbs2[:])
                hp = sp.tile([P, KC, W2N], BF16, tag="hw")
                normalize(hp, lambda s: xrb[:, s, :], bcs2[:], bbs2[:])
                # ---- fc1 -> gelu ----------------------------------------
                for m1 in range(M1T):
                    w1t = spw.tile([P, KC, P], BF16, tag="wt")
                    nc.sync.dma_start(w1t[:], w1c[m1])
                    ps1 = accp.tile([P, W2N], F32, tag="acc")
                    for k in range(KC):
                        nc.tensor.matmul(ps1[:], w1t[:, k, :], hp[:, k, :],
                                         start=(k == 0), stop=(k == KC - 1))
                    nc.scalar.activation(h2a[:, m1, :], ps1[:],
                                         AF.Gelu_apprx_tanh,
                                         bias=f1bs[:, m1:m1 + 1])
                # ---- next pair's LN1 normalize (overlaps fc1/fc2) --------
                if pr + 1 < NPAIR:
                    xp = load_xpair(pr + 1)
                    hw = sp.tile([P, KC, W2N], BF16, tag="hw")
                    normalize(hw, lambda s, xp=xp: x_slice(xp, s),
                              bcs[:, pr + 1, :], bbs[:, pr + 1, :])
                # ---- fc2 + residual + output -----------------------------
                M1Q = M1T // 4
                for pm in range(KC):
                    ps2 = accp.tile([P, W2N], F32, tag="acc")
                    for qt in range(4):
                        w2t = spw2.tile([P, M1Q, P], BF16, tag="w2t")
                        nc.sync.dma_start(w2t[:],
                                          w2[pm, :, M1Q * qt:M1Q * (qt + 1), :])
                        for m1 in range(M1Q):
                            m1g = M1Q * qt + m1
                            nc.tensor.matmul(ps2[:], w2t[:, m1, :],
                                             h2a[:, m1g, :],
                                             start=(m1g == 0), stop=False)
                    nc.tensor.matmul(
                        ps2[:], bias2[:1, C + P * pm:C + P * (pm + 1)],
                        ones_b[:1, :W2N], start=False, stop=True)
                    ot = spr.tile([P, 2, N], F32, tag="ot")
                    nc.vector.tensor_add(
                        ot[:].rearrange("p u n -> p (u n)"),
                        xrb[:, pm, :], ps2[:])
                    for wh in range(2):
                        nc.sync.dma_start(outT[w0 + wh, :, pm, :],
                                          ot[:, wh, :])

    nc.compile()
    return nc


# ---------------------------------------------------------------------------
# host side
# ---------------------------------------------------------------------------

def _qk_colmap():
    m = np.full(2 * H * HS, -1, np.int64)
    for h in range(H):
        m[HS * h:HS * h + DH] = np.arange(72 * h, 72 * h + 72)
        m[H * HS + HS * h:H * HS + HS * h + DH] = \
            np.arange(C + 72 * h, C + 72 * h + 72)
    return m


def _prep_core_inputs(x_c, c_c, wdict):
    """x_c: [nw, N, C], c_c: [nw, C] -> per-core input map"""
    nw = x_c.shape[0]
    xT = np.ascontiguousarray(
        x_c.transpose(0, 2, 1).reshape(nw, KC, P, N).transpose(
            0, 2, 1, 3)).astype(NPBF16)
    caug = np.zeros((nw, 1280), np.float32)
    caug[:, :C] = c_c
    caug[:, C] = 1.0
    cT = np.ascontiguousarray(caug.T.reshape(10, P, nw)).astype(NPBF16)
    return {"xb": xT, "cT": cT, **wdict}


def _prep_weights(qkv_w, qkv_b, qkvt_w, qkvt_b, rpb_table, rel_idx,
                  proj_w, proj_b, fc1_w, fc1_b, fc2_w, fc2_b):
    qkmap = _qk_colmap()
    amap = np.concatenate([qkmap, np.arange(2 * C, 3 * C)])  # 4224 cols
    valid = amap >= 0

    wct = np.zeros((1280, 4224), np.float32)
    wct[:C, valid] = qkvt_w[amap[valid], :].T
    wct[C, valid] = (qkv_b + qkvt_b)[amap[valid]]
    wct = wct.reshape(10, P, 4224).astype(NPBF16)

    nqk = 2 * H * HS
    wqkT = np.zeros((C, nqk), np.float32)
    wqkT[:, valid[:nqk]] = qkv_w[qkmap[valid[:nqk]], :].T
    wqk = np.ascontiguousarray(
        wqkT.reshape(KC, P, QKM, P).transpose(2, 1, 0, 3)).astype(NPBF16)

    wv = np.ascontiguousarray(
        qkv_w[2 * C:, :].T.reshape(KC, P, 4, 288).transpose(
            2, 1, 0, 3)).astype(NPBF16)

    bias = rpb_table[rel_idx]                      # [N(n), N(m), H]
    expb = np.ascontiguousarray(
        np.exp(bias).transpose(2, 1, 0).reshape(H, 2, P, N).transpose(
            0, 2, 1, 3))
    expb2 = np.ascontiguousarray(
        np.concatenate([expb, expb], axis=-1)).astype(NPBF16)

    wp_sl = np.zeros((P, H, C), np.float32)        # [slot-row d, head, p]
    for h in range(H):
        wp_sl[1:73, h, :] = proj_w[:, 72 * h:72 * h + 72].T
    wps = np.ascontiguousarray(
        wp_sl.reshape(P, H, KC, P).transpose(2, 0, 1, 3)).astype(NPBF16)

    w1c = np.ascontiguousarray(
        fc1_w.T.reshape(KC, P, M1T, P).transpose(2, 1, 0, 3)).astype(NPBF16)
    w2 = np.ascontiguousarray(
        fc2_w.T.reshape(M1T, P, KC, P).transpose(2, 1, 0, 3)).astype(NPBF16)
    f1b = np.ascontiguousarray(fc1_b.reshape(M1T, P).T).astype(np.float32)
    b2 = np.concatenate([proj_b, fc2_b]).reshape(1, 2 * C).astype(NPBF16)
    wpc = np.zeros((2, W2N), NPBF16)
    wpc[0, :N] = 1.0
    wpc[1, N:] = 1.0

    return {"wct": wct, "wqk": wqk, "wv": wv, "expb2": expb2, "wps": wps,
            "w1c": w1c, "w2": w2, "f1b": f1b, "b2": b2, "wpc": wpc}


_PROGRAM = None


def kernel(x, c, qkv_w, qkv_b, qkvt_w, qkvt_b, rpb_table, proj_w, proj_b,
           fc1_w, fc1_b, fc2_w, fc2_b, rel_idx, _trace=False):
    global _PROGRAM
    x = np.asarray(x, np.float32)
    c = np.asarray(c, np.float32)
    wdict = _prep_weights(
        np.asarray(qkv_w, np.float32), np.asarray(qkv_b, np.float32),
        np.asarray(qkvt_w, np.float32), np.asarray(qkvt_b, np.float32),
        np.asarray(rpb_table, np.float32), np.asarray(rel_idx),
        np.asarray(proj_w, np.float32), np.asarray(proj_b, np.float32),
        np.asarray(fc1_w, np.float32), np.asarray(fc1_b, np.float32),
        np.asarray(fc2_w, np.float32), np.asarray(fc2_b, np.float32))

    if _PROGRAM is None:
        _PROGRAM = build_program(NW)
    nc = _PROGRAM

    in_maps = []
    for core in range(NCORES):
        sl = slice(core * NW, (core + 1) * NW)
        in_maps.append(_prep_core_inputs(x[sl], c[sl], wdict))

    res = bass_utils.run_bass_kernel_spmd(
        nc, in_maps, core_ids=list(range(NCORES)), trace=_trace)

    out = np.empty((B, N, C), np.float32)
    for core in range(NCORES):
        oT = res.results[core]["outT"]            # [NW, P, KC, N]
        out[core * NW:(core + 1) * NW] = \
            oT.transpose(0, 2, 1, 3).reshape(NW, C, N).transpose(0, 2, 1)
    if _trace:
        return out, res
    return out


# revision 54
# speedup vs baseline: 1.2547x; 1.0312x over previous
"""DiffiT transformer block kernel for 8 Trainium2 NeuronCores.

Data-parallel over the B=64 window axis (8 windows per core). Activations
are feature-major ([channel, token]); every linear contracts over the SBUF
partition axis. Redesign v2 for tensor-engine density:

- x enters once as bf16 and stays resident in SBUF; the attention residual
  is kept on-chip (no DRAM roundtrip for the mid-residual).
- LN1 stats + broadcast tiles for ALL window pairs are computed in a
  prologue; each pair's input normalize (vector work) is emitted during the
  previous pair's fc1 so the PE never waits on LN at pair start.
- LN2 stat matmuls are interleaved into the proj loop with one-chunk lag.
- Softmax: per-head scores for both windows share one [128,512] PSUM tile,
  exp'd in one activation; the rel-pos bias multiply runs on gpsimd; the
  1/l normalize uses a PE ones-matmul broadcast + fused vector multiply
  (PSUM x PSUM -> SBUF bf16), emitted with one-head lookahead.
- O is packed tight feature-major, so proj contracts 9 (not 16) k-tiles.
- Rank-1 fixups (biases, time-token conditioning) fold into the PSUM
  accumulations as K=1/K=2 matmuls.
"""

import math
from contextlib import ExitStack

import numpy as np
import ml_dtypes

import concourse.bass as bass
import concourse.mybir as mybir
import concourse.tile as tile
from concourse import bacc
from concourse import bass_utils

F32 = mybir.dt.float32
BF16 = mybir.dt.bfloat16
NPBF16 = ml_dtypes.bfloat16
AF = mybir.ActivationFunctionType

P = 128
WS = 16
N = 256            # tokens per window
C = 1152           # hidden
H = 16             # heads
DH = 72            # head dim
HS = 96            # head stride in the QK packing (32-aligned, >= DH)
MLP = 4608
EPS = 1e-6
B = 64
NCORES = 8
NW = B // NCORES   # windows per core
KC = C // P        # 9  k-tiles over the hidden dim
QKM = 2 * H * HS // P   # 24 m-tiles over packed Q+K (96-stride)
KOFF = QKM // 2    # first K-side m-tile
M1T = MLP // P     # 36 fc1 row tiles
SCALE = 1.0 / math.sqrt(DH)
NPAIR = NW // 2
W2N = 2 * N        # tokens per window pair


def _qk_pieces(h):
    """32-aligned partition pieces covering head h's 72 rows in the
    96-stride packing: [(subtile, base, length), ...]; piece legality:
    base 0 any len, base 64 len<=64, base 32/96 len<=32."""
    start, end = HS * h, HS * h + DH
    out = []
    while start < end:
        sub, base = divmod(start, P)
        ln = min(end - start, P - base)
        if base == 64:
            ln = min(ln, 64)
        elif base in (32, 96):
            ln = min(ln, 32)
        elif base != 0:
            raise AssertionError(base)
        out.append((sub, base, ln))
        start += ln
    return out


def _ost_pieces(h):
    """pieces of the tight 72-row span of head h in the [P, KC, *] packing:
    [(subtile, partition base, src offset, length), ...]"""
    f0 = DH * h
    out = []
    off = 0
    while off < DH:
        sub, pb = divmod(f0 + off, P)
        ln = min(DH - off, P - pb)
        out.append((sub, pb, off, ln))
        off += ln
    return out


def build_program(nw=NW):
    nc = bacc.Bacc("TRN2", target_bir_lowering=False, debug=False,
                   num_devices=NCORES)

    # register the layernorm epsilon as a const AP (activation float biases
    # other than 0.0/1.0 need one), same pattern as Bass.__init__
    eps_t = nc.alloc_sbuf_tensor("const-eps", [P, 1], F32)
    nc.gpsimd.memset(eps_t.ap(), EPS)
    nc.const_aps.aps[(F32, EPS)] = eps_t.ap()
    nc.all_engine_barrier()

    def din(name, shape, dt):
        return nc.dram_tensor(name, shape, dt, kind="ExternalInput").ap()

    xb = din("xb", [nw, P, KC, N], BF16)         # x, feature-major bf16
    cT = din("cT", [10, P, nw], BF16)            # c augmented with ones row
    wct = din("wct", [10, P, 4224], BF16)        # qkvt^T reordered + bias row
    wqk = din("wqk", [QKM, P, KC, P], BF16)      # qkv^T QK part, 96-stride
    wv = din("wv", [4, P, KC, 288], BF16)        # qkv^T V part, chunk-major
    expb = din("expb", [H, P, 2, N], BF16)       # exp(rel-pos bias)^T per head
    wps = din("wps", [KC, P, H, P], BF16)        # proj^T, head-slot padded
    w1c = din("w1c", [M1T, P, KC, P], BF16)      # fc1^T pre-chunked
    w2 = din("w2", [KC, P, M1T, P], BF16)        # fc2^T, pm-chunked
    f1b = din("f1b", [P, M1T], F32)              # fc1 bias, per-partition
    b2 = din("b2", [1, 2 * C], BF16)             # proj_b ++ fc2_b
    wpc = din("wpc", [2, W2N], BF16)             # window indicator rows
    outT = nc.dram_tensor("outT", [nw, P, KC, N], F32,
                          kind="ExternalOutput").ap()

    with tile.TileContext(nc) as tc, ExitStack() as ctx:
        keep = ctx.enter_context(tc.tile_pool(name="keep", bufs=1))
        dram = ctx.enter_context(tc.tile_pool(name="dram", bufs=1,
                                              space="DRAM"))

        ones_b = keep.tile([1, W2N], BF16, tag="ones_b")
        ones_c = keep.tile([P, 1], BF16, tag="ones_c")
        wpair = keep.tile([2, W2N], BF16, tag="wpair")   # window indicators
        nc.gpsimd.memset(ones_b[:], 1.0)
        nc.gpsimd.memset(ones_c[:], 1.0)
        nc.sync.dma_start(wpair[:], wpc[:])
        bias2 = keep.tile([1, 2 * C], BF16, tag="bias2")
        nc.sync.dma_start(bias2[:], b2[:])
        f1bs = keep.tile([P, M1T], F32, tag="f1bs")
        nc.sync.dma_start(f1bs[:], f1b[:])

        # resident activations / packings
        bcs = keep.tile([P, NPAIR, W2N], BF16, tag="bcs")    # LN1 rstd bcast
        bbs = keep.tile([P, NPAIR, W2N], BF16, tag="bbs")    # LN1 -m*r bcast
        qkst = keep.tile([P, QKM, W2N], BF16, tag="qkst")
        vsl = keep.tile([P, 2, 2, H, P], BF16, tag="vsl")    # padded V slots
        ost = keep.tile([P, H, W2N], BF16, tag="ost")        # O head slots
        xrb = keep.tile([P, KC, W2N], BF16, tag="xrb")       # x + attn (bf16)
        h2a = keep.tile([P, M1T, W2N], BF16, tag="h2a")
        bcs2 = keep.tile([P, W2N], BF16, tag="bcs2")         # LN2 bcasts
        bbs2 = keep.tile([P, W2N], BF16, tag="bbs2")

        tdram = dram.tile([nw, 4224], BF16)

        nc.vector.memset(vsl[:].rearrange("p a b h c -> p (a b h c)"), 0.0)
        nc.vector.memset(vsl[:, :, :, :, 0:1], 1.0)
        nc.vector.memset(ost[64:, :, :].rearrange("p h n -> p (h n)"), 0.0)

        # ---- phase 0: conditioning T = c_aug @ W_ct ----------------------
        with tc.tile_pool(name="ph0", bufs=2) as p0, \
             tc.tile_pool(name="ph0p", bufs=2, space="PSUM") as pp0:
            caug = p0.tile([P, 10, nw], BF16, tag="caug")
            nc.sync.dma_start(caug[:], cT.rearrange("k p w -> p k w"))
            tsb = p0.tile([8, 4224], BF16, tag="tsb")
            for i in range(9):
                n0, nl = i * 512, min(512, 4224 - i * 512)
                tps = pp0.tile([8, 512], F32, tag="tps")
                for k in range(10):
                    wt = p0.tile([P, 512], BF16, tag="wctt")
                    nc.sync.dma_start(wt[:, :nl], wct[k, :, n0:n0 + nl])
                    nc.tensor.matmul(tps[:nw, :nl], caug[:, k, :], wt[:, :nl],
                                     start=(k == 0), stop=(k == 9))
                nc.scalar.activation(tsb[:nw, n0:n0 + nl], tps[:nw, :nl],
                                     AF.Copy)
            nc.sync.dma_start(tdram[:, :], tsb[:nw, :])

        with tc.tile_pool(name="sp", bufs=2) as sp, \
             tc.tile_pool(name="spw", bufs=3) as spw, \
             tc.tile_pool(name="spv", bufs=2) as spv, \
             tc.tile_pool(name="spw2", bufs=3) as spw2, \
             tc.tile_pool(name="sps", bufs=2) as sps, \
             tc.tile_pool(name="spp", bufs=2) as spp, \
             tc.tile_pool(name="spr", bufs=2) as spr, \
             tc.tile_pool(name="spx", bufs=1) as spx, \
             tc.tile_pool(name="spt", bufs=1) as spt, \
             tc.tile_pool(name="rows", bufs=1) as rows, \
             tc.tile_pool(name="accp", bufs=6, space="PSUM") as accp, \
             tc.tile_pool(name="statp", bufs=1, space="PSUM") as statp:

            def x_slice(xp, s):
                return xp[:, s, :, :].rearrange("p u n -> p (u n)")

            def load_xpair(pr):
                xp = spx.tile([P, KC, 2, N], BF16, tag="xp")
                for wh in range(2):
                    nc.sync.dma_start(xp[:, :, wh, :], xb[2 * pr + wh])
                return xp

            # LN stats for a pair from a fetch(s) -> [P, W2N] bf16 source.
            # Returns (rstd, bneg) bf16 [1, W2N] row tiles.
            def ln_rows(fetch, tag):
                ms0 = statp.tile([1, W2N], F32, tag="ms0")
                ms1 = statp.tile([1, W2N], F32, tag="ms1")
                for s in range(KC):
                    xbs = fetch(s)
                    xsq = sp.tile([P, W2N], BF16, tag="xsq")
                    nc.vector.tensor_mul(xsq[:], xbs, xbs)
                    nc.tensor.matmul(ms0[:], ones_c[:], xbs,
                                     start=(s == 0), stop=(s == KC - 1))
                    nc.tensor.matmul(ms1[:], ones_c[:], xsq[:],
                                     start=(s == 0), stop=(s == KC - 1))
                return ln_rows_tail(ms0, ms1, tag)

            def ln_rows_tail(ms0, ms1, tag):
                mean = rows.tile([1, W2N], F32, tag="r_mean")
                ra = rows.tile([1, W2N], F32, tag="r_a")
                rb = rows.tile([1, W2N], F32, tag="r_b")
                nc.vector.tensor_scalar_mul(mean[:], ms0[:], 1.0 / C)
                nc.vector.tensor_scalar_mul(ra[:], ms1[:], 1.0 / C)
                nc.vector.tensor_mul(rb[:], mean[:], mean[:])
                nc.vector.tensor_sub(ra[:], ra[:], rb[:])      # var
                nc.scalar.activation(rb[:], ra[:], AF.Sqrt, bias=EPS)
                nc.vector.reciprocal_approx_fast(ra[:], rb[:])
                rstd = rows.tile([1, W2N], BF16, tag="r_rstd")
                nc.vector.tensor_copy(rstd[:], ra[:])
                bneg = rows.tile([1, W2N], BF16, tag="r_bneg")
                nc.vector.scalar_tensor_tensor(
                    bneg[:], mean[:], -1.0, rstd[:],
                    mybir.AluOpType.mult, mybir.AluOpType.mult)
                return rstd, bneg

            def bcast_rows(rstd, bneg, dst_c, dst_b):
                bcp = accp.tile([P, W2N], F32, tag="acc")
                nc.tensor.matmul(bcp[:], ones_b[:1, :P], rstd[:],
                                 start=True, stop=True)
                bbp = accp.tile([P, W2N], F32, tag="acc")
                nc.tensor.matmul(bbp[:], ones_b[:1, :P], bneg[:],
                                 start=True, stop=True)
                nc.scalar.activation(dst_c, bcp[:], AF.Copy)
                nc.scalar.activation(dst_b, bbp[:], AF.Copy)

            # normalize: dst[:, s, :] = src(s) * bc + bb, alternating engines
            def normalize(dst, fetch, bc, bb):
                for s in range(KC):
                    eng = nc.vector if s % 2 == 0 else nc.gpsimd
                    eng.tensor_mul(dst[:, s, :], fetch(s), bc)
                    eng.tensor_add(dst[:, s, :], dst[:, s, :], bb)

            # ---- prologue: LN1 stats/broadcasts for all pairs ------------
            # pair 0 last so its x stays resident for the main loop
            for pr in [1, 2, 3, 0][-NPAIR:]:
                xp = load_xpair(pr)
                rstd, bneg = ln_rows(
                    lambda s, xp=xp: x_slice(xp, s), f"l1_{pr}")
                bcast_rows(rstd, bneg, bcs[:, pr, :], bbs[:, pr, :])

            hw = sp.tile([P, KC, W2N], BF16, tag="hw")
            normalize(hw, lambda s: x_slice(xp, s),
                      bcs[:, 0, :], bbs[:, 0, :])

            # =================== main loop over pairs =====================
            for pr in range(NPAIR):
                w0 = 2 * pr
                # ---- QK (96-stride packed) -------------------------------
                t2all = spt.tile([2, QKM * P], BF16, tag="t2all")
                nc.sync.dma_start(t2all[:], tdram[w0:w0 + 2, :QKM * P])
                for m in range(QKM):
                    wt = spw.tile([P, KC, P], BF16, tag="wt")
                    nc.sync.dma_start(wt[:], wqk[m])
                    qs = accp.tile([P, W2N], F32, tag="acc")
                    for k in range(KC):
                        nc.tensor.matmul(qs[:], wt[:, k, :], hw[:, k, :],
                                         start=(k == 0), stop=False)
                    nc.tensor.matmul(qs[:], t2all[:, P * m:P * (m + 1)],
                                     wpair[:], start=False, stop=True)
                    nc.scalar.activation(qkst[:, m, :], qs[:], AF.Copy)
                # ---- V token-major into padded per-head slots ------------
                for nch in range(4):
                    wvt = spv.tile([P, KC, 288], BF16, tag="wvt")
                    nc.sync.dma_start(wvt[:], wv[nch])
                    t1vc = spr.tile([1, 2, 288], BF16, tag="t1v")
                    nc.sync.dma_start(
                        t1vc[:],
                        tdram[w0:w0 + 2, 3072 + 288 * nch:3072 + 288 * (nch + 1)]
                        .unsqueeze(0))
                    for tch in range(4):
                        wh, ms = divmod(tch, 2)
                        vs = accp.tile([P, W2N], F32, tag="acc")
                        tsl = slice(tch * P, (tch + 1) * P)
                        for k in range(KC):
                            nc.tensor.matmul(vs[:, :288], hw[:, k, tsl],
                                             wvt[:, k, :],
                                             start=(k == 0), stop=False)
                        nc.tensor.matmul(
                            vs[:, :288], ones_b[:1, :P], t1vc[:1, wh, :],
                            start=False, stop=True)
                        nc.scalar.activation(
                            vsl[:, wh, ms, 4 * nch:4 * nch + 4, 1:73],
                            vs[:, :288].rearrange("p (h d) -> p h d", d=72),
                            AF.Copy)
                # ---- attention, one-head lookahead normalize -------------
                def emit_norm(h, pt):
                    for wh in range(2):
                        polb = accp.tile([P, W2N], F32, tag="acc")
                        for ms in range(2):
                            nc.tensor.matmul(
                                polb[:, :N], vsl[:, wh, ms, h, :],
                                pt[:, ms, wh * N:(wh + 1) * N],
                                start=(ms == 0), stop=(ms == 1))
                        linv32 = spr.tile([1, N], F32, tag="linv32")
                        nc.vector.reciprocal_approx_fast(linv32[:],
                                                         polb[0:1, :N])
                        linv = spr.tile([1, N], BF16, tag="linv")
                        nc.vector.tensor_copy(linv[:], linv32[:])
                        nc.tensor.matmul(polb[:73, N:], ones_b[:1, :73],
                                         linv[:], start=True, stop=True)
                        obf = spr.tile([P, N], BF16, tag="obf")
                        nc.vector.tensor_copy(obf[:73, :], polb[:73, :N])
                        nc.vector.tensor_mul(
                            ost[:73, h, wh * N:(wh + 1) * N],
                            obf[:73, :], polb[:73, N:])

                prev = None
                for h in range(H):
                    et = sps.tile([P, 2, N], BF16, tag="ebt")
                    nc.sync.dma_start(et[:], expb[h])
                    pt = spp.tile([P, 2, W2N], BF16, tag="pt")
                    qp = _qk_pieces(h)
                    for ms in range(2):
                        ssp = accp.tile([P, W2N], F32, tag="acc")
                        for wh in range(2):
                            msl = slice(wh * N + ms * P, wh * N + (ms + 1) * P)
                            nsl = slice(wh * N, (wh + 1) * N)
                            for i, (sub, base, ln) in enumerate(qp):
                                nc.tensor.matmul(
                                    ssp[:, nsl],
                                    qkst[base:base + ln, KOFF + sub, msl],
                                    qkst[base:base + ln, sub, nsl],
                                    start=(i == 0),
                                    stop=(i == len(qp) - 1),
                                    tile_position=(base, 0))
                        nc.scalar.activation(pt[:, ms, :], ssp[:], AF.Exp,
                                             scale=SCALE)
                        nc.gpsimd.tensor_mul(
                            pt[:, ms, :].rearrange("p (u n) -> p u n", n=N),
                            pt[:, ms, :].rearrange("p (u n) -> p u n", n=N),
                            et[:, ms:ms + 1, :].to_broadcast((P, 2, N)))
                    if prev is not None:
                        emit_norm(*prev)
                    prev = (h, pt)
                emit_norm(*prev)
                # ---- proj + residual + interleaved LN2 stats -------------
                ms0 = statp.tile([1, W2N], F32, tag="ms0")
                ms1 = statp.tile([1, W2N], F32, tag="ms1")

                def emit_stats(s):
                    xsq = sp.tile([P, W2N], BF16, tag="xsq")
                    nc.vector.tensor_mul(xsq[:], xrb[:, s, :], xrb[:, s, :])
                    nc.tensor.matmul(ms0[:], ones_c[:], xrb[:, s, :],
                                     start=(s == 0), stop=(s == KC - 1))
                    nc.tensor.matmul(ms1[:], ones_c[:], xsq[:],
                                     start=(s == 0), stop=(s == KC - 1))

                HH = H // 2
                for pc in range(KC):
                    yps = accp.tile([P, W2N], F32, tag="acc")
                    for half in range(2):
                        wpt = spw2.tile([P, HH, P], BF16, tag="wpt")
                        nc.sync.dma_start(
                            wpt[:], wps[pc, :, HH * half:HH * (half + 1), :])
                        for hh in range(HH):
                            hg = HH * half + hh
                            nc.tensor.matmul(yps[:], wpt[:, hh, :],
                                             ost[:, hg, :],
                                             start=(hg == 0), stop=False)
                    nc.tensor.matmul(yps[:], bias2[:1, P * pc:P * (pc + 1)],
                                     ones_b[:1, :W2N], start=False, stop=True)
                    nc.vector.tensor_add(xrb[:, pc, :], x_slice(xp, pc),
                                         yps[:])
                    if pc > 1:
                        emit_stats(pc - 2)
                emit_stats(KC - 2)
                emit_stats(KC - 1)
                rstd2, bneg2 = ln_rows_tail(ms0, ms1, "l2")
                bcast_rows(rstd2, bneg2, bcs2[:], bbs2[:])
                hp = sp.tile([P, KC, W2N], BF16, tag="hw")
                normalize(hp, lambda s: xrb[:, s, :], bcs2[:], bbs2[:])
                # ---- fc1 -> gelu ----------------------------------------
                for m1 in range(M1T):
                    w1t = spw.tile([P, KC, P], BF16, tag="wt")
                    nc.sync.dma_start(w1t[:], w1c[m1])
                    ps1 = accp.tile([P, W2N], F32, tag="acc")
                    for k in range(KC):
                        nc.tensor.matmul(ps1[:], w1t[:, k, :], hp[:, k, :],
                                         start=(k == 0), stop=(k == KC - 1))
                    nc.scalar.activation(h2a[:, m1, :], ps1[:],
                                         AF.Gelu_apprx_tanh,
                                         bias=f1bs[:, m1:m1 + 1])
                # ---- next pair's LN1 normalize (overlaps fc1/fc2) --------
                if pr + 1 < NPAIR:
                    xp = load_xpair(pr + 1)
                    hw = sp.tile([P, KC, W2N], BF16, tag="hw")
                    normalize(hw, lambda s, xp=xp: x_slice(xp, s),
                              bcs[:, pr + 1, :], bbs[:, pr + 1, :])
                # ---- fc2 + residual + output -----------------------------
                M1Q = M1T // 4
                for pm in range(KC):
                    ps2 = accp.tile([P, W2N], F32, tag="acc")
                    for qt in range(4):
                        w2t = spw2.tile([P, M1Q, P], BF16, tag="w2t")
                        nc.sync.dma_start(w2t[:],
                                          w2[pm, :, M1Q * qt:M1Q * (qt + 1), :])
                        for m1 in range(M1Q):
                            m1g = M1Q * qt + m1
                            nc.tensor.matmul(ps2[:], w2t[:, m1, :],
                                             h2a[:, m1g, :],
                                             start=(m1g == 0), stop=False)
                    nc.tensor.matmul(
                        ps2[:], bias2[:1, C + P * pm:C + P * (pm + 1)],
                        ones_b[:1, :W2N], start=False, stop=True)
                    ot = spr.tile([P, 2, N], F32, tag="ot")
                    nc.vector.tensor_add(
                        ot[:].rearrange("p u n -> p (u n)"),
                        xrb[:, pm, :], ps2[:])
                    for wh in range(2):
                        nc.sync.dma_start(outT[w0 + wh, :, pm, :],
                                          ot[:, wh, :])

    nc.compile()
    return nc


# ---------------------------------------------------------------------------
# host side
# ---------------------------------------------------------------------------

def _qk_colmap():
    m = np.full(2 * H * HS, -1, np.int64)
    for h in range(H):
        m[HS * h:HS * h + DH] = np.arange(72 * h, 72 * h + 72)
        m[H * HS + HS * h:H * HS + HS * h + DH] = \
            np.arange(C + 72 * h, C + 72 * h + 72)
    return m


def _prep_core_inputs(x_c, c_c, wdict):
    """x_c: [nw, N, C], c_c: [nw, C] -> per-core input map"""
    nw = x_c.shape[0]
    xT = np.ascontiguousarray(
        x_c.transpose(0, 2, 1).reshape(nw, KC, P, N).transpose(
            0, 2, 1, 3)).astype(NPBF16)
    caug = np.zeros((nw, 1280), np.float32)
    caug[:, :C] = c_c
    caug[:, C] = 1.0
    cT = np.ascontiguousarray(caug.T.reshape(10, P, nw)).astype(NPBF16)
    return {"xb": xT, "cT": cT, **wdict}


def _prep_weights(qkv_w, qkv_b, qkvt_w, qkvt_b, rpb_table, rel_idx,
                  proj_w, proj_b, fc1_w, fc1_b, fc2_w, fc2_b):
    qkmap = _qk_colmap()
    amap = np.concatenate([qkmap, np.arange(2 * C, 3 * C)])  # 4224 cols
    valid = amap >= 0

    wct = np.zeros((1280, 4224), np.float32)
    wct[:C, valid] = qkvt_w[amap[valid], :].T
    wct[C, valid] = (qkv_b + qkvt_b)[amap[valid]]
    wct = wct.reshape(10, P, 4224).astype(NPBF16)

    nqk = 2 * H * HS
    wqkT = np.zeros((C, nqk), np.float32)
    wqkT[:, valid[:nqk]] = qkv_w[qkmap[valid[:nqk]], :].T
    wqk = np.ascontiguousarray(
        wqkT.reshape(KC, P, QKM, P).transpose(2, 1, 0, 3)).astype(NPBF16)

    wv = np.ascontiguousarray(
        qkv_w[2 * C:, :].T.reshape(KC, P, 4, 288).transpose(
            2, 1, 0, 3)).astype(NPBF16)

    bias = rpb_table[rel_idx]                      # [N(n), N(m), H]
    expb = np.ascontiguousarray(
        np.exp(bias).transpose(2, 1, 0).reshape(H, 2, P, N).transpose(
            0, 2, 1, 3)).astype(NPBF16)

    wp_sl = np.zeros((P, H, C), np.float32)        # [slot-row d, head, p]
    for h in range(H):
        wp_sl[1:73, h, :] = proj_w[:, 72 * h:72 * h + 72].T
    wps = np.ascontiguousarray(
        wp_sl.reshape(P, H, KC, P).transpose(2, 0, 1, 3)).astype(NPBF16)

    w1c = np.ascontiguousarray(
        fc1_w.T.reshape(KC, P, M1T, P).transpose(2, 1, 0, 3)).astype(NPBF16)
    w2 = np.ascontiguousarray(
        fc2_w.T.reshape(M1T, P, KC, P).transpose(2, 1, 0, 3)).astype(NPBF16)
    f1b = np.ascontiguousarray(fc1_b.reshape(M1T, P).T).astype(np.float32)
    b2 = np.concatenate([proj_b, fc2_b]).reshape(1, 2 * C).astype(NPBF16)
    wpc = np.zeros((2, W2N), NPBF16)
    wpc[0, :N] = 1.0
    wpc[1, N:] = 1.0

    return {"wct": wct, "wqk": wqk, "wv": wv, "expb": expb, "wps": wps,
            "w1c": w1c, "w2": w2, "f1b": f1b, "b2": b2, "wpc": wpc}


_PROGRAM = None


def kernel(x, c, qkv_w, qkv_b, qkvt_w, qkvt_b, rpb_table, proj_w, proj_b,
           fc1_w, fc1_b, fc2_w, fc2_b, rel_idx, _trace=False):
    global _PROGRAM
    x = np.asarray(x, np.float32)
    c = np.asarray(c, np.float32)
    wdict = _prep_weights(
        np.asarray(qkv_w, np.float32), np.asarray(qkv_b, np.float32),
        np.asarray(qkvt_w, np.float32), np.asarray(qkvt_b, np.float32),
        np.asarray(rpb_table, np.float32), np.asarray(rel_idx),
        np.asarray(proj_w, np.float32), np.asarray(proj_b, np.float32),
        np.asarray(fc1_w, np.float32), np.asarray(fc1_b, np.float32),
        np.asarray(fc2_w, np.float32), np.asarray(fc2_b, np.float32))

    if _PROGRAM is None:
        _PROGRAM = build_program(NW)
    nc = _PROGRAM

    in_maps = []
    for core in range(NCORES):
        sl = slice(core * NW, (core + 1) * NW)
        in_maps.append(_prep_core_inputs(x[sl], c[sl], wdict))

    res = bass_utils.run_bass_kernel_spmd(
        nc, in_maps, core_ids=list(range(NCORES)), trace=_trace)

    out = np.empty((B, N, C), np.float32)
    for core in range(NCORES):
        oT = res.results[core]["outT"]            # [NW, P, KC, N]
        out[core * NW:(core + 1) * NW] = \
            oT.transpose(0, 2, 1, 3).reshape(NW, C, N).transpose(0, 2, 1)
    if _trace:
        return out, res
    return out


# revision 67
# speedup vs baseline: 1.4539x; 1.1587x over previous
"""DiffiT transformer block kernel for 8 Trainium2 NeuronCores.

Data-parallel over the B=64 window axis (8 windows per core). Activations
are feature-major ([channel, token]); every linear contracts over the SBUF
partition axis. Redesign v2 for tensor-engine density:

- x enters once as bf16 and stays resident in SBUF; the attention residual
  is kept on-chip (no DRAM roundtrip for the mid-residual).
- LN1 stats + broadcast tiles for ALL window pairs are computed in a
  prologue; each pair's input normalize (vector work) is emitted during the
  previous pair's fc1 so the PE never waits on LN at pair start.
- LN2 stat matmuls are interleaved into the proj loop with one-chunk lag.
- Softmax: per-head scores for both windows share one [128,512] PSUM tile,
  exp'd in one activation; the rel-pos bias multiply runs on gpsimd; the
  1/l normalize uses a PE ones-matmul broadcast + fused vector multiply
  (PSUM x PSUM -> SBUF bf16), emitted with one-head lookahead.
- O is packed tight feature-major, so proj contracts 9 (not 16) k-tiles.
- Rank-1 fixups (biases, time-token conditioning) fold into the PSUM
  accumulations as K=1/K=2 matmuls.
"""

import math
from contextlib import ExitStack

import numpy as np
import ml_dtypes

import concourse.bass as bass
import concourse.mybir as mybir
import concourse.tile as tile
from concourse import bacc
from concourse import bass_utils

F32 = mybir.dt.float32
BF16 = mybir.dt.bfloat16
NPBF16 = ml_dtypes.bfloat16
AF = mybir.ActivationFunctionType

P = 128
WS = 16
N = 256            # tokens per window
C = 1152           # hidden
H = 16             # heads
DH = 72            # head dim
HS = 96            # head stride in the QK packing (32-aligned, >= DH)
MLP = 4608
EPS = 1e-6
B = 64
NCORES = 8
NW = B // NCORES   # windows per core
KC = C // P        # 9  k-tiles over the hidden dim
QKM = 2 * H * HS // P   # 24 m-tiles over packed Q+K (96-stride)
KOFF = QKM // 2    # first K-side m-tile
M1T = MLP // P     # 36 fc1 row tiles
SCALE = 1.0 / math.sqrt(DH)
NPAIR = NW // 2
W2N = 2 * N        # tokens per window pair


def _qk_pieces(h):
    """32-aligned partition pieces covering head h's 72 rows in the
    96-stride packing: [(subtile, base, length), ...]; piece legality:
    base 0 any len, base 64 len<=64, base 32/96 len<=32."""
    start, end = HS * h, HS * h + DH
    out = []
    while start < end:
        sub, base = divmod(start, P)
        ln = min(end - start, P - base)
        if base == 64:
            ln = min(ln, 64)
        elif base in (32, 96):
            ln = min(ln, 32)
        elif base != 0:
            raise AssertionError(base)
        out.append((sub, base, ln))
        start += ln
    return out


def _ost_pieces(h):
    """pieces of the tight 72-row span of head h in the [P, KC, *] packing:
    [(subtile, partition base, src offset, length), ...]"""
    f0 = DH * h
    out = []
    off = 0
    while off < DH:
        sub, pb = divmod(f0 + off, P)
        ln = min(DH - off, P - pb)
        out.append((sub, pb, off, ln))
        off += ln
    return out


def build_program(nw=NW):
    nc = bacc.Bacc("TRN2", target_bir_lowering=False, debug=False,
                   num_devices=NCORES)

    # register the layernorm epsilon as a const AP (activation float biases
    # other than 0.0/1.0 need one), same pattern as Bass.__init__
    eps_t = nc.alloc_sbuf_tensor("const-eps", [P, 1], F32)
    nc.gpsimd.memset(eps_t.ap(), EPS)
    nc.const_aps.aps[(F32, EPS)] = eps_t.ap()
    nc.all_engine_barrier()

    def din(name, shape, dt):
        return nc.dram_tensor(name, shape, dt, kind="ExternalInput").ap()

    xb = din("xb", [nw, P, KC, N], BF16)         # x, feature-major bf16
    cT = din("cT", [10, P, nw], BF16)            # c augmented with ones row
    wct = din("wct", [10, P, 4224], BF16)        # qkvt^T reordered + bias row
    wqk = din("wqk", [QKM, P, KC, P], BF16)      # qkv^T QK part, 96-stride
    wv = din("wv", [4, P, KC, 288], BF16)        # qkv^T V part, chunk-major
    expb = din("expb", [H, P, 2, N], BF16)       # exp(rel-pos bias)^T per head
    wps = din("wps", [KC, P, H, P], BF16)        # proj^T, head-slot padded
    w1c = din("w1c", [M1T, P, KC, P], BF16)      # fc1^T pre-chunked
    w2 = din("w2", [KC, P, M1T, P], BF16)        # fc2^T, pm-chunked
    f1b = din("f1b", [P, M1T], F32)              # fc1 bias, per-partition
    b2 = din("b2", [1, 2 * C], BF16)             # proj_b ++ fc2_b
    wpc = din("wpc", [2, W2N], BF16)             # window indicator rows
    outT = nc.dram_tensor("outT", [nw, P, KC, N], BF16,
                          kind="ExternalOutput").ap()

    with tile.TileContext(nc) as tc, ExitStack() as ctx:
        keep = ctx.enter_context(tc.tile_pool(name="keep", bufs=1))
        dram = ctx.enter_context(tc.tile_pool(name="dram", bufs=1,
                                              space="DRAM"))

        ones_b = keep.tile([1, W2N], BF16, tag="ones_b")
        ones_c = keep.tile([P, 1], BF16, tag="ones_c")
        wpair = keep.tile([2, W2N], BF16, tag="wpair")   # window indicators
        nc.gpsimd.memset(ones_b[:], 1.0)
        nc.gpsimd.memset(ones_c[:], 1.0)
        nc.sync.dma_start(wpair[:], wpc[:])
        bias2 = keep.tile([1, 2 * C], BF16, tag="bias2")
        nc.sync.dma_start(bias2[:], b2[:])
        f1bs = keep.tile([P, M1T], F32, tag="f1bs")
        nc.sync.dma_start(f1bs[:], f1b[:])

        # resident activations / packings
        bcs = keep.tile([P, NPAIR, W2N], BF16, tag="bcs")    # LN1 rstd bcast
        bbs = keep.tile([P, NPAIR, W2N], BF16, tag="bbs")    # LN1 -m*r bcast
        qkst = keep.tile([P, QKM, W2N], BF16, tag="qkst")
        vsl = keep.tile([P, 2, 2, H, P], BF16, tag="vsl")    # padded V slots
        ost = keep.tile([P, H, W2N], BF16, tag="ost")        # O head slots
        xrb = keep.tile([P, KC, W2N], BF16, tag="xrb")       # x + attn (bf16)
        h2a = keep.tile([P, M1T, W2N], BF16, tag="h2a")
        bcs2 = keep.tile([P, W2N], BF16, tag="bcs2")         # LN2 bcasts
        bbs2 = keep.tile([P, W2N], BF16, tag="bbs2")

        tdram = dram.tile([nw, 4224], BF16)

        nc.vector.memset(vsl[:].rearrange("p a b h c -> p (a b h c)"), 0.0)
        nc.vector.memset(vsl[:, :, :, :, 0:1], 1.0)
        nc.vector.memset(ost[64:, :, :].rearrange("p h n -> p (h n)"), 0.0)

        # ---- pool-parameterized LN helpers ------------------------------
        def ln_stats_pair(xw0, xw1, xsqp, stp):
            ms0 = stp.tile([1, W2N], F32, tag="ms0")
            ms1 = stp.tile([1, W2N], F32, tag="ms1")
            for s in range(KC):
                for wh, xw in ((0, xw0), (1, xw1)):
                    nsl = slice(wh * N, (wh + 1) * N)
                    xsq = xsqp.tile([P, N], BF16, tag="xsq")
                    nc.vector.tensor_mul(xsq[:], xw[:, s, :], xw[:, s, :])
                    nc.tensor.matmul(ms0[:, nsl], ones_c[:], xw[:, s, :],
                                     start=(s == 0), stop=(s == KC - 1))
                    nc.tensor.matmul(ms1[:, nsl], ones_c[:], xsq[:],
                                     start=(s == 0), stop=(s == KC - 1))
            return ms0, ms1

        def ln_rows_tail(ms0, ms1, rowsp):
            mean = rowsp.tile([1, W2N], F32, tag="r_mean")
            ra = rowsp.tile([1, W2N], F32, tag="r_a")
            rb = rowsp.tile([1, W2N], F32, tag="r_b")
            nc.vector.tensor_scalar_mul(mean[:], ms0[:], 1.0 / C)
            nc.vector.tensor_scalar_mul(ra[:], ms1[:], 1.0 / C)
            nc.vector.tensor_mul(rb[:], mean[:], mean[:])
            nc.vector.tensor_sub(ra[:], ra[:], rb[:])      # var
            nc.scalar.activation(rb[:], ra[:], AF.Sqrt, bias=EPS)
            nc.vector.reciprocal_approx_fast(ra[:], rb[:])
            rstd = rowsp.tile([1, W2N], BF16, tag="r_rstd")
            nc.vector.tensor_copy(rstd[:], ra[:])
            bneg = rowsp.tile([1, W2N], BF16, tag="r_bneg")
            nc.vector.scalar_tensor_tensor(
                bneg[:], mean[:], -1.0, rstd[:],
                mybir.AluOpType.mult, mybir.AluOpType.mult)
            return rstd, bneg

        def bcast_rows(rstd, bneg, dst_c, dst_b, psp):
            bcp = psp.tile([P, W2N], F32, tag="acc")
            nc.tensor.matmul(bcp[:], ones_b[:1, :P], rstd[:],
                             start=True, stop=True)
            bbp = psp.tile([P, W2N], F32, tag="acc")
            nc.tensor.matmul(bbp[:], ones_b[:1, :P], bneg[:],
                             start=True, stop=True)
            nc.scalar.activation(dst_c, bcp[:], AF.Copy)
            nc.scalar.activation(dst_b, bbp[:], AF.Copy)

        def normalize_w(dst, xw0, xw1, bc, bb):
            for s in range(KC):
                for wh, xw in ((0, xw0), (1, xw1)):
                    eng = nc.vector if wh == 0 else nc.gpsimd
                    nsl = slice(wh * N, (wh + 1) * N)
                    eng.tensor_mul(dst[:, s, nsl], xw[:, s, :], bc[:, nsl])
                    eng.tensor_add(dst[:, s, nsl], dst[:, s, nsl], bb[:, nsl])

        # ---- phase 0: LN1 stats (pairs 1..) + conditioning --------------
        with tc.tile_pool(name="ph0", bufs=1) as p0, \
             tc.tile_pool(name="ph0x", bufs=2) as p0x, \
             tc.tile_pool(name="ph0r", bufs=1) as p0r, \
             tc.tile_pool(name="ph0q", bufs=2) as p0q, \
             tc.tile_pool(name="ph0p", bufs=2, space="PSUM") as pp0, \
             tc.tile_pool(name="ph0s", bufs=1, space="PSUM") as pp0s:
            caug = p0.tile([P, 10, nw], BF16, tag="caug")
            nc.sync.dma_start(caug[:], cT.rearrange("k p w -> p k w"))
            for pr in range(1, NPAIR):
                xw0 = p0x.tile([P, KC, N], BF16, tag="xw")
                nc.sync.dma_start(xw0[:], xb[2 * pr])
                xw1 = p0x.tile([P, KC, N], BF16, tag="xw")
                nc.sync.dma_start(xw1[:], xb[2 * pr + 1])
                ms0, ms1 = ln_stats_pair(xw0, xw1, p0q, pp0s)
                rstd, bneg = ln_rows_tail(ms0, ms1, p0r)
                bcast_rows(rstd, bneg, bcs[:, pr, :], bbs[:, pr, :], pp0)
            # conditioning T = c_aug @ W_ct, two batched weight halves
            tsb = p0.tile([8, 4224], BF16, tag="tsb")
            CH = 2112
            for hlf in range(2):
                wcth = p0.tile([P, 10, CH], BF16, tag="wcth")
                nc.sync.dma_start(
                    wcth[:], wct[:, :, CH * hlf:CH * (hlf + 1)]
                    .rearrange("k p c -> p k c"))
                for c0 in range(0, CH, 512):
                    nl = min(512, CH - c0)
                    tps = pp0.tile([8, 512], F32, tag="tps")
                    for k in range(10):
                        nc.tensor.matmul(tps[:nw, :nl], caug[:, k, :],
                                         wcth[:, k, c0:c0 + nl],
                                         start=(k == 0), stop=(k == 9))
                    nc.scalar.activation(
                        tsb[:nw, CH * hlf + c0:CH * hlf + c0 + nl],
                        tps[:nw, :nl], AF.Copy)
                nc.sync.dma_start(tdram[:, CH * hlf:CH * (hlf + 1)],
                                  tsb[:nw, CH * hlf:CH * (hlf + 1)])

        with tc.tile_pool(name="sp", bufs=2) as sp, \
             tc.tile_pool(name="spw", bufs=3) as spw, \
             tc.tile_pool(name="spv", bufs=2) as spv, \
             tc.tile_pool(name="spw2", bufs=3) as spw2, \
             tc.tile_pool(name="sps", bufs=2) as sps, \
             tc.tile_pool(name="spp", bufs=3) as spp, \
             tc.tile_pool(name="spr", bufs=2) as spr, \
             tc.tile_pool(name="spx", bufs=2) as spx, \
             tc.tile_pool(name="spt", bufs=1) as spt, \
             tc.tile_pool(name="rows", bufs=1) as rows, \
             tc.tile_pool(name="accp", bufs=6, space="PSUM") as accp, \
             tc.tile_pool(name="statp", bufs=1, space="PSUM") as statp:

            def load_xwin(w):
                xw = spx.tile([P, KC, N], BF16, tag="xw")
                nc.sync.dma_start(xw[:], xb[w])
                return xw

            # ---- pair 0 LN1 + first normalize ----------------------------
            xw0 = load_xwin(0)
            xw1 = load_xwin(1)
            ms0, ms1 = ln_stats_pair(xw0, xw1, sp, statp)
            rstd, bneg = ln_rows_tail(ms0, ms1, rows)
            bcast_rows(rstd, bneg, bcs[:, 0, :], bbs[:, 0, :], accp)
            hw = sp.tile([P, KC, W2N], BF16, tag="hw")
            normalize_w(hw, xw0, xw1, bcs[:, 0, :], bbs[:, 0, :])

            # =================== main loop over pairs =====================
            for pr in range(NPAIR):
                w0 = 2 * pr
                # ---- QK (96-stride packed) -------------------------------
                t2all = spt.tile([2, QKM * P], BF16, tag="t2all")
                nc.sync.dma_start(t2all[:], tdram[w0:w0 + 2, :QKM * P])
                for m in range(QKM):
                    wt = spw.tile([P, KC, P], BF16, tag="wt")
                    nc.sync.dma_start(wt[:], wqk[m])
                    qs = accp.tile([P, W2N], F32, tag="acc")
                    for k in range(KC):
                        nc.tensor.matmul(qs[:], wt[:, k, :], hw[:, k, :],
                                         start=(k == 0), stop=False)
                    nc.tensor.matmul(qs[:], t2all[:, P * m:P * (m + 1)],
                                     wpair[:], start=False, stop=True)
                    nc.scalar.activation(qkst[:, m, :], qs[:], AF.Copy)
                # ---- V token-major into padded per-head slots ------------
                for nch in range(4):
                    wvt = spv.tile([P, KC, 288], BF16, tag="wvt")
                    nc.sync.dma_start(wvt[:], wv[nch])
                    t1vc = spr.tile([1, 2, 288], BF16, tag="t1v")
                    nc.sync.dma_start(
                        t1vc[:],
                        tdram[w0:w0 + 2, 3072 + 288 * nch:3072 + 288 * (nch + 1)]
                        .unsqueeze(0))
                    for tch in range(4):
                        wh, ms = divmod(tch, 2)
                        vs = accp.tile([P, W2N], F32, tag="acc")
                        tsl = slice(tch * P, (tch + 1) * P)
                        for k in range(KC):
                            nc.tensor.matmul(vs[:, :288], hw[:, k, tsl],
                                             wvt[:, k, :],
                                             start=(k == 0), stop=False)
                        nc.tensor.matmul(
                            vs[:, :288], ones_b[:1, :P], t1vc[:1, wh, :],
                            start=False, stop=True)
                        nc.scalar.activation(
                            vsl[:, wh, ms, 4 * nch:4 * nch + 4, 1:73],
                            vs[:, :288].rearrange("p (h d) -> p h d", d=72),
                            AF.Copy)
                # ---- attention, one-head lookahead normalize -------------
                def emit_norm(h, pt):
                    for wh in range(2):
                        polb = accp.tile([P, W2N], F32, tag="acc")
                        for ms in range(2):
                            nc.tensor.matmul(
                                polb[:, :N], vsl[:, wh, ms, h, :],
                                pt[:, ms, wh * N:(wh + 1) * N],
                                start=(ms == 0), stop=(ms == 1))
                        linv32 = spr.tile([1, N], F32, tag="linv32")
                        nc.vector.reciprocal_approx_fast(linv32[:],
                                                         polb[0:1, :N])
                        linv = spr.tile([1, N], BF16, tag="linv")
                        nc.vector.tensor_copy(linv[:], linv32[:])
                        nc.tensor.matmul(polb[:73, N:], ones_b[:1, :73],
                                         linv[:], start=True, stop=True)
                        obf = spr.tile([P, N], BF16, tag="obf")
                        nc.vector.tensor_copy(obf[:73, :], polb[:73, :N])
                        nc.vector.tensor_mul(
                            ost[:73, h, wh * N:(wh + 1) * N],
                            obf[:73, :], polb[:73, N:])

                pend = []
                for h in range(H):
                    et = sps.tile([P, 2, N], BF16, tag="ebt")
                    nc.sync.dma_start(et[:], expb[h])
                    pt = spp.tile([P, 2, W2N], BF16, tag="pt")
                    qp = _qk_pieces(h)
                    for ms in range(2):
                        ssp = accp.tile([P, W2N], F32, tag="acc")
                        for wh in range(2):
                            msl = slice(wh * N + ms * P, wh * N + (ms + 1) * P)
                            nsl = slice(wh * N, (wh + 1) * N)
                            for i, (sub, base, ln) in enumerate(qp):
                                nc.tensor.matmul(
                                    ssp[:, nsl],
                                    qkst[base:base + ln, KOFF + sub, msl],
                                    qkst[base:base + ln, sub, nsl],
                                    start=(i == 0),
                                    stop=(i == len(qp) - 1),
                                    tile_position=(base, 0))
                        nc.scalar.activation(pt[:, ms, :], ssp[:], AF.Exp,
                                             scale=SCALE)
                        nc.gpsimd.tensor_mul(
                            pt[:, ms, :].rearrange("p (u n) -> p u n", n=N),
                            pt[:, ms, :].rearrange("p (u n) -> p u n", n=N),
                            et[:, ms:ms + 1, :].to_broadcast((P, 2, N)))
                    if len(pend) == 2:
                        emit_norm(*pend.pop(0))
                    pend.append((h, pt))
                for it in pend:
                    emit_norm(*it)
                # ---- proj + residual + interleaved LN2 stats -------------
                ms0 = statp.tile([1, W2N], F32, tag="ms0")
                ms1 = statp.tile([1, W2N], F32, tag="ms1")

                def emit_stats(s):
                    xsq = sp.tile([P, W2N], BF16, tag="xsq2")
                    nc.vector.tensor_mul(xsq[:], xrb[:, s, :], xrb[:, s, :])
                    nc.tensor.matmul(ms0[:], ones_c[:], xrb[:, s, :],
                                     start=(s == 0), stop=(s == KC - 1))
                    nc.tensor.matmul(ms1[:], ones_c[:], xsq[:],
                                     start=(s == 0), stop=(s == KC - 1))

                HH = H // 2
                for pc in range(KC):
                    yps = accp.tile([P, W2N], F32, tag="acc")
                    for half in range(2):
                        wpt = spw2.tile([P, HH, P], BF16, tag="wpt")
                        nc.sync.dma_start(
                            wpt[:], wps[pc, :, HH * half:HH * (half + 1), :])
                        for hh in range(HH):
                            hg = HH * half + hh
                            nc.tensor.matmul(yps[:], wpt[:, hh, :],
                                             ost[:, hg, :],
                                             start=(hg == 0), stop=False)
                    nc.tensor.matmul(yps[:], bias2[:1, P * pc:P * (pc + 1)],
                                     ones_b[:1, :W2N], start=False, stop=True)
                    nc.vector.tensor_add(xrb[:, pc, 0:N], xw0[:, pc, :],
                                         yps[:, 0:N])
                    nc.vector.tensor_add(xrb[:, pc, N:], xw1[:, pc, :],
                                         yps[:, N:])
                    if pc > 1:
                        emit_stats(pc - 2)
                emit_stats(KC - 2)
                emit_stats(KC - 1)
                rstd2, bneg2 = ln_rows_tail(ms0, ms1, rows)
                bcast_rows(rstd2, bneg2, bcs2[:], bbs2[:], accp)
                hp = sp.tile([P, KC, W2N], BF16, tag="hw")
                for s in range(KC):
                    eng = nc.vector if s % 2 == 0 else nc.gpsimd
                    eng.tensor_mul(hp[:, s, :], xrb[:, s, :], bcs2[:])
                    eng.tensor_add(hp[:, s, :], hp[:, s, :], bbs2[:])
                # ---- fc1 -> gelu ----------------------------------------
                for m1 in range(M1T):
                    w1t = spw.tile([P, KC, P], BF16, tag="wt")
                    nc.sync.dma_start(w1t[:], w1c[m1])
                    ps1 = accp.tile([P, W2N], F32, tag="acc")
                    for k in range(KC):
                        nc.tensor.matmul(ps1[:], w1t[:, k, :], hp[:, k, :],
                                         start=(k == 0), stop=(k == KC - 1))
                    nc.scalar.activation(h2a[:, m1, :], ps1[:],
                                         AF.Gelu_apprx_tanh,
                                         bias=f1bs[:, m1:m1 + 1])
                # ---- next pair's LN1 normalize (overlaps fc1/fc2) --------
                if pr + 1 < NPAIR:
                    xw0 = load_xwin(2 * pr + 2)
                    xw1 = load_xwin(2 * pr + 3)
                    hw = sp.tile([P, KC, W2N], BF16, tag="hw")
                    normalize_w(hw, xw0, xw1,
                                bcs[:, pr + 1, :], bbs[:, pr + 1, :])
                # ---- fc2 + residual + output -----------------------------
                M1Q = M1T // 4
                for pm in range(KC):
                    ps2 = accp.tile([P, W2N], F32, tag="acc")
                    for qt in range(4):
                        w2t = spw2.tile([P, M1Q, P], BF16, tag="w2t")
                        nc.sync.dma_start(w2t[:],
                                          w2[pm, :, M1Q * qt:M1Q * (qt + 1), :])
                        for m1 in range(M1Q):
                            m1g = M1Q * qt + m1
                            nc.tensor.matmul(ps2[:], w2t[:, m1, :],
                                             h2a[:, m1g, :],
                                             start=(m1g == 0), stop=False)
                    nc.tensor.matmul(
                        ps2[:], bias2[:1, C + P * pm:C + P * (pm + 1)],
                        ones_b[:1, :W2N], start=False, stop=True)
                    ot = spr.tile([P, 2, N], BF16, tag="ot")
                    nc.vector.tensor_add(
                        ot[:].rearrange("p u n -> p (u n)"),
                        xrb[:, pm, :], ps2[:])
                    for wh in range(2):
                        nc.sync.dma_start(outT[w0 + wh, :, pm, :],
                                          ot[:, wh, :])

    nc.compile()
    return nc


# ---------------------------------------------------------------------------
# host side
# ---------------------------------------------------------------------------

def _qk_colmap():
    m = np.full(2 * H * HS, -1, np.int64)
    for h in range(H):
        m[HS * h:HS * h + DH] = np.arange(72 * h, 72 * h + 72)
        m[H * HS + HS * h:H * HS + HS * h + DH] = \
            np.arange(C + 72 * h, C + 72 * h + 72)
    return m


def _prep_core_inputs(x_c, c_c, wdict):
    """x_c: [nw, N, C], c_c: [nw, C] -> per-core input map"""
    nw = x_c.shape[0]
    xT = np.ascontiguousarray(
        x_c.transpose(0, 2, 1).reshape(nw, KC, P, N).transpose(
            0, 2, 1, 3)).astype(NPBF16)
    caug = np.zeros((nw, 1280), np.float32)
    caug[:, :C] = c_c
    caug[:, C] = 1.0
    cT = np.ascontiguousarray(caug.T.reshape(10, P, nw)).astype(NPBF16)
    return {"xb": xT, "cT": cT, **wdict}


def _prep_weights(qkv_w, qkv_b, qkvt_w, qkvt_b, rpb_table, rel_idx,
                  proj_w, proj_b, fc1_w, fc1_b, fc2_w, fc2_b):
    qkmap = _qk_colmap()
    amap = np.concatenate([qkmap, np.arange(2 * C, 3 * C)])  # 4224 cols
    valid = amap >= 0

    wct = np.zeros((1280, 4224), np.float32)
    wct[:C, valid] = qkvt_w[amap[valid], :].T
    wct[C, valid] = (qkv_b + qkvt_b)[amap[valid]]
    wct = wct.reshape(10, P, 4224).astype(NPBF16)

    nqk = 2 * H * HS
    wqkT = np.zeros((C, nqk), np.float32)
    wqkT[:, valid[:nqk]] = qkv_w[qkmap[valid[:nqk]], :].T
    wqk = np.ascontiguousarray(
        wqkT.reshape(KC, P, QKM, P).transpose(2, 1, 0, 3)).astype(NPBF16)

    wv = np.ascontiguousarray(
        qkv_w[2 * C:, :].T.reshape(KC, P, 4, 288).transpose(
            2, 1, 0, 3)).astype(NPBF16)

    bias = rpb_table[rel_idx]                      # [N(n), N(m), H]
    expb = np.ascontiguousarray(
        np.exp(bias).transpose(2, 1, 0).reshape(H, 2, P, N).transpose(
            0, 2, 1, 3)).astype(NPBF16)

    wp_sl = np.zeros((P, H, C), np.float32)        # [slot-row d, head, p]
    for h in range(H):
        wp_sl[1:73, h, :] = proj_w[:, 72 * h:72 * h + 72].T
    wps = np.ascontiguousarray(
        wp_sl.reshape(P, H, KC, P).transpose(2, 0, 1, 3)).astype(NPBF16)

    w1c = np.ascontiguousarray(
        fc1_w.T.reshape(KC, P, M1T, P).transpose(2, 1, 0, 3)).astype(NPBF16)
    w2 = np.ascontiguousarray(
        fc2_w.T.reshape(M1T, P, KC, P).transpose(2, 1, 0, 3)).astype(NPBF16)
    f1b = np.ascontiguousarray(fc1_b.reshape(M1T, P).T).astype(np.float32)
    b2 = np.concatenate([proj_b, fc2_b]).reshape(1, 2 * C).astype(NPBF16)
    wpc = np.zeros((2, W2N), NPBF16)
    wpc[0, :N] = 1.0
    wpc[1, N:] = 1.0

    return {"wct": wct, "wqk": wqk, "wv": wv, "expb": expb, "wps": wps,
            "w1c": w1c, "w2": w2, "f1b": f1b, "b2": b2, "wpc": wpc}


_PROGRAM = None


def kernel(x, c, qkv_w, qkv_b, qkvt_w, qkvt_b, rpb_table, proj_w, proj_b,
           fc1_w, fc1_b, fc2_w, fc2_b, rel_idx, _trace=False):
    global _PROGRAM
    x = np.asarray(x, np.float32)
    c = np.asarray(c, np.float32)
    wdict = _prep_weights(
        np.asarray(qkv_w, np.float32), np.asarray(qkv_b, np.float32),
        np.asarray(qkvt_w, np.float32), np.asarray(qkvt_b, np.float32),
        np.asarray(rpb_table, np.float32), np.asarray(rel_idx),
        np.asarray(proj_w, np.float32), np.asarray(proj_b, np.float32),
        np.asarray(fc1_w, np.float32), np.asarray(fc1_b, np.float32),
        np.asarray(fc2_w, np.float32), np.asarray(fc2_b, np.float32))

    if _PROGRAM is None:
        _PROGRAM = build_program(NW)
    nc = _PROGRAM

    in_maps = []
    for core in range(NCORES):
        sl = slice(core * NW, (core + 1) * NW)
        in_maps.append(_prep_core_inputs(x[sl], c[sl], wdict))

    res = bass_utils.run_bass_kernel_spmd(
        nc, in_maps, core_ids=list(range(NCORES)), trace=_trace)

    out = np.empty((B, N, C), np.float32)
    for core in range(NCORES):
        oT = res.results[core]["outT"].astype(np.float32)  # [NW, P, KC, N]
        out[core * NW:(core + 1) * NW] = \
            oT.transpose(0, 2, 1, 3).reshape(NW, C, N).transpose(0, 2, 1)
    if _trace:
        return out, res
    return out
